# revision 3
# baseline (speedup 1.0000x reference)
"""AttnBlock (GroupNorm -> QKV 1x1 -> HxW self-attention -> proj -> residual)
as a Bass/Tile kernel on 8 TRN2 NeuronCores.

Sharding: data-parallel over batch B=2 and sequence-parallel over HW
quarters (4 cores per image, 1024 queries each). Each core redundantly
computes GroupNorm + full-image keys/values so there is no cross-core
communication. The host rolls the pixel axis per core so each core's
query quarter starts at pixel 0 (GN stats and attention-over-keys are
permutation invariant), letting all cores run one SPMD program.

Algebraic folds (host side) eliminate whole on-device phases:
  S   = q.k = (Wq xn + qb).(Wk xn + kb)
      = xn^T (Wq^T Wk) xn + (Wk^T qb).xn_j + [terms const over keys j,
        which softmax cancels]
    so with M = Wq^T Wk and u0 = Wk^T qb folded on host:
        qm = M^T xn_q + u0   (replaces the Q AND K projections)
        S  = qm^T xn         (keys are raw xn -- no K tensor at all)
  out = Wp (V P) + pb, V = Wv xn + vb, P = softmax rows
      = (Wp Wv) (xn P) + (pw vb + pb)
    so with Wpv = Wp Wv and kc0 = pw vb + pb folded on host:
        vt' = xn^T Wpv^T     (one value pass instead of V-proj + out-proj)
        out^T[i,:] = sum_j P[j,i] vt'[j,:]  -- the PV matmul emits the
        FINAL projected output directly in [query, channel] layout, so
        the epilogue is just a per-partition 1/D scale + residual add.
This removes ~1.3G MACs/core of matmul and ~100 PSUM->SBUF copy ops.

Precision: matmul operands in fp8e4 (E4M3) with DoubleRow perf mode;
fp32 PSUM accumulation. Folded weights are pre-scaled x256 on the host
(fp8 normal range); the 1/256 folds into existing psum->SBUF copies.
Softmax skips max-subtraction (logits in ~[-1.5, 1.5] by construction);
the softmax division is a per-partition scale in the epilogue.

Engine balance: exp runs in batched [128, 2x512] ops on ScalarE; the
GN-apply cast and the PSUM->SBUF copies are split across Vector/Scalar/
GpSimd; residual adds ride the otherwise idle GpSimd engine.
"""

import sys

sys.path.insert(0, "/opt/trn_rl_repo")

import numpy as np
import ml_dtypes

B, C, H, W = 2, 512, 64, 64
N = H * W            # 4096 pixels per image
NQ = N // 4          # 1024 queries per core
CI = C // 128        # 4 channel chunks of 128
NUM_GROUPS = 32
EPS = 1e-6
P = 128
FD = 512             # matmul moving free dim
JT = N // P          # 32 key tiles
IC = NQ // FD        # 2 query chunks of 512
IT = NQ // P         # 8 query tiles of 128
SCALE = float(C) ** -0.5
WS = 256.0           # host-side weight pre-scale (keeps fp8e4 in normal range)

F8 = ml_dtypes.float8_e4m3


def build_bass():
    import concourse.bass as bass
    import concourse.tile as tile
    import concourse.mybir as mybir
    from concourse import bacc
    from contextlib import ExitStack

    f32 = mybir.dt.float32
    f8 = mybir.dt.float8e4
    AF = mybir.ActivationFunctionType
    OP = mybir.AluOpType
    DR = mybir.MatmulPerfMode.DoubleRow

    nc = bacc.Bacc("TRN2")

    # ---------------- DRAM I/O ----------------
    x_img = nc.dram_tensor("x_img", [P, CI, N], f32, kind="ExternalInput")
    x_resT = nc.dram_tensor("x_resT", [P, IT, C], f32, kind="ExternalInput")
    mT = nc.dram_tensor("mT", [P, CI, C], f8, kind="ExternalInput")
    wpvT = nc.dram_tensor("wpvT", [P, CI, C], f8, kind="ExternalInput")
    u0c = nc.dram_tensor("u0c", [P, CI], f32, kind="ExternalInput")
    kc0_bc = nc.dram_tensor("kc0_bc", [P, C], f32, kind="ExternalInput")
    gns_t = nc.dram_tensor("gns_t", [P, CI], f32, kind="ExternalInput")
    gnb_t = nc.dram_tensor("gnb_t", [P, CI], f32, kind="ExternalInput")
    g_red = nc.dram_tensor("g_red", [P, 8], f32, kind="ExternalInput")
    g_bc = nc.dram_tensor("g_bc", [8, P], f32, kind="ExternalInput")
    out_t = nc.dram_tensor("out_t", [P, IT, C], f32, kind="ExternalOutput")

    with tile.TileContext(nc) as tc, ExitStack() as top:
        consts = top.enter_context(tc.tile_pool(name="consts", bufs=1))
        big = top.enter_context(tc.tile_pool(name="big", bufs=1))
        smallp = top.enter_context(tc.tile_pool(name="smallp", bufs=1))
        outst = top.enter_context(tc.tile_pool(name="outst", bufs=4))

        # x DMAs first — GroupNorm stats are the critical-path head, so x
        # must not queue behind the constant loads
        gnscope = ExitStack()
        xresid = gnscope.enter_context(tc.tile_pool(name="xresid", bufs=1))
        xr = [xresid.tile([P, N], f32, name=f"xr{ci}") for ci in range(CI)]
        for ci in range(CI):
            for h in range(2):
                nc.sync.dma_start(
                    xr[ci][:, h * 2048:(h + 1) * 2048],
                    x_img[:, ci, h * 2048:(h + 1) * 2048],
                )

        # ---- load constants ----
        # tiny GroupNorm constants first: the per-chunk reduce chain needs
        # them early, and they'd otherwise queue behind 9MB of x + weights
        gns_s = consts.tile([P, CI], f32)
        gnb_s = consts.tile([P, CI], f32)
        nc.sync.dma_start(gns_s, gns_t[:])
        nc.sync.dma_start(gnb_s, gnb_t[:])
        gr_s = consts.tile([P, 8], f32)
        gb_s = consts.tile([8, P], f32)
        nc.sync.dma_start(gr_s, g_red[:])
        nc.sync.dma_start(gb_s, g_bc[:])
        m_s = consts.tile([P, CI, C], f8)
        wpv_s = consts.tile([P, CI, C], f8)
        nc.sync.dma_start(m_s, mT[:])
        nc.sync.dma_start(wpv_s, wpvT[:])
        u0_s = consts.tile([P, CI], f32)
        nc.sync.dma_start(u0_s, u0c[:])
        kc0_s = consts.tile([P, C], f32)
        nc.sync.dma_start(kc0_s, kc0_bc[:])
        # padded to 16 so the DoubleRow pair-plane stride is 16B (%16 rule)
        ones2 = consts.tile([P, 2, 16], f8)
        nc.vector.memset(ones2, 1.0)
        ones_1 = consts.tile([1, 1], f32)
        nc.vector.memset(ones_1, 1.0)
        eps8 = consts.tile([8, 1], f32)
        nc.vector.memset(eps8, EPS)

        xres_s = big.tile([P, IT, C], f32)
        nc.sync.dma_start(xres_s, x_resT[:])

        # residual pre-adds (x + kc0 const row) on the otherwise-idle
        # GpSimd engine (SBUF-only); runs in the DMA-bound prelude
        for it in range(IT):
            nc.gpsimd.tensor_add(xres_s[:, it, :], xres_s[:, it, :], kc0_s)

        # big persistent tensors (fp8)
        xn = big.tile([P, CI, N], f8)            # normalized x (keys AND gn)
        vt_sb = big.tile([P, JT, C], f8)         # vt' = xn^T Wpv^T, [j, o]
        qm_sb = big.tile([P, CI, NQ], f8)        # qm = M^T xn_q + u0, [d, i]
        # exp(S) for one query-half, persisted so the softmax-denominator
        # reduction can run batched at the half boundary (off the hot loop)
        pexpall = big.tile([P, JT // 2, 2, FD], f8)

        # =============== Phase 1: GroupNorm ===============
        gnw = gnscope.enter_context(tc.tile_pool(name="gnw", bufs=1))
        gnps = gnscope.enter_context(
            tc.tile_pool(name="gnps", bufs=1, space="PSUM")
        )

        mv2 = gnw.tile([P, 2 * CI], f32)  # per-channel (mean, m2) per chunk
        gps = gnps.tile([8, 2 * CI], f32, tag="g")
        gst = gnw.tile([8, 2 * CI], f32)
        bcps = gnps.tile([P, 2 * CI], f32, tag="bc")
        a_all = gnw.tile([P, CI], f32)
        b_all = gnw.tile([P, CI], f32)
        for ci in range(CI):
            bnst = gnw.tile([P, 8, 6], f32, tag="bnst", bufs=2)
            for s in range(8):
                nc.vector.bn_stats(
                    bnst[:, s, :], xr[ci][:, s * 512:(s + 1) * 512]
                )
            nc.vector.bn_aggr(mv2[:, 2 * ci:2 * ci + 2], bnst)
            # per-chunk reduce chain on ScalarE (idle during stats), so it
            # never steals VectorE from the stats stream; chunks 0..2 finish
            # under later chunks' DMA/stats and only chunk 3's chain is on
            # the critical path. Column math via activation scale/bias APs;
            # only the reciprocal must run on VectorE.
            mu = mv2[:, 2 * ci:2 * ci + 1]
            m2 = mv2[:, 2 * ci + 1:2 * ci + 2]
            sq = gnw.tile([P, 1], f32, tag="sq", bufs=2)
            nc.scalar.activation(sq, mu, AF.Copy, scale=mu)      # mean^2
            nc.scalar.activation(m2, sq, AF.Identity, bias=m2)   # 2nd moment
            nc.tensor.matmul(
                gps[:, 2 * ci:2 * ci + 2], lhsT=gr_s,
                rhs=mv2[:, 2 * ci:2 * ci + 2], start=True, stop=True,
            )
            gmu = gst[:, 2 * ci:2 * ci + 1]
            gm2 = gst[:, 2 * ci + 1:2 * ci + 2]
            nc.scalar.copy(gst[:, 2 * ci:2 * ci + 2], gps[:, 2 * ci:2 * ci + 2])
            gsq = gnw.tile([8, 1], f32, tag="gsq", bufs=2)
            nc.scalar.activation(gsq, gmu, AF.Copy, scale=gmu)   # gmean^2
            nc.scalar.activation(gm2, gsq, AF.Identity, scale=-1.0, bias=gm2)
            nc.scalar.activation(gm2, gm2, AF.Sqrt, bias=eps8)   # std
            nc.vector.reciprocal(gm2, gm2)                       # rstd
            nc.tensor.matmul(
                bcps[:, 2 * ci:2 * ci + 2], lhsT=gb_s,
                rhs=gst[:, 2 * ci:2 * ci + 2], start=True, stop=True,
            )
            a = a_all[:, ci:ci + 1]
            b = b_all[:, ci:ci + 1]
            chp = gnw.tile([P, 2], f32, tag="chp", bufs=2)
            nc.scalar.copy(chp, bcps[:, 2 * ci:2 * ci + 2])
            nc.scalar.activation(a, gns_s[:, ci:ci + 1], AF.Copy,
                                 scale=chp[:, 1:2])              # rstd*gns
            tmpc = gnw.tile([P, 1], f32, tag="tmpc", bufs=2)
            nc.scalar.activation(tmpc, chp[:, 0:1], AF.Copy, scale=a)
            nc.scalar.activation(b, tmpc, AF.Identity, scale=-1.0,
                                 bias=gnb_s[:, ci:ci + 1])

        def emit_apply(h, engines):
            # xn[:, ci, h-quarter] = a*x + b, split across engines
            for ci in range(CI):
                dst = xn[:, ci, h * 1024:(h + 1) * 1024]
                src = xr[ci][:, h * 1024:(h + 1) * 1024]
                eng = engines[ci % len(engines)]
                if eng == "act":
                    nc.scalar.activation(
                        dst, src, AF.Identity,
                        bias=b_all[:, ci:ci + 1], scale=a_all[:, ci:ci + 1],
                    )
                elif eng == "pool":
                    nc.gpsimd.tensor_scalar(
                        dst, src,
                        a_all[:, ci:ci + 1], b_all[:, ci:ci + 1],
                        OP.mult, OP.add,
                    )
                else:
                    nc.vector.tensor_scalar(
                        dst, src,
                        a_all[:, ci:ci + 1], b_all[:, ci:ci + 1],
                        OP.mult, OP.add,
                    )

        # quarter 0 first (it holds the queries + first key tiles); fan the
        # chunk casts across all three elementwise engines
        emit_apply(0, ["vec", "act", "pool", "vec"])
        gnscope.close()

        # psum pools for attention: 2 double-size rotating tiles (s-tiles,
        # vt'/qm production, d) + 4 accumulator banks for the out^T tiles
        ph = ExitStack()
        mxp = ph.enter_context(tc.tile_pool(name="mxp", bufs=2, space="PSUM"))
        outp = ph.enter_context(tc.tile_pool(name="outp", bufs=1, space="PSUM"))

        def emit_qm(co, icq):
            # qm[d-block co, i-chunk icq] = M^T xn_q + u0
            ps = mxp.tile([P, 2, FD], f32, tag="mx", name=f"qm{co}_{icq}")
            pss = ps[:, 0, :]
            for ep in range(CI // 2):
                nc.tensor.matmul(
                    pss,
                    lhsT=m_s[:, 2 * ep:2 * ep + 2, co * P:(co + 1) * P],
                    rhs=xn[:, 2 * ep:2 * ep + 2, icq * FD:(icq + 1) * FD],
                    start=(ep == 0),
                    stop=(ep == CI // 2 - 1),
                    perf_mode=DR,
                )
            nc.scalar.activation(
                qm_sb[:, co, icq * FD:(icq + 1) * FD], pss, AF.Identity,
                bias=u0_s[:, co:co + 1], scale=1.0 / WS,
            )

        def emit_vt(jt, act=False):
            # vt'[j-tile jt, :] = xn^T Wpv^T
            ps = mxp.tile([P, 2, FD], f32, tag="mx", name=f"vt{jt}")
            pss = ps[:, 0, :]
            for ep in range(CI // 2):
                nc.tensor.matmul(
                    pss,
                    lhsT=xn[:, 2 * ep:2 * ep + 2, jt * P:(jt + 1) * P],
                    rhs=wpv_s[:, 2 * ep:2 * ep + 2, :],
                    start=(ep == 0),
                    stop=(ep == CI // 2 - 1),
                    perf_mode=DR,
                )
            if act:
                nc.scalar.activation(
                    vt_sb[:, jt, :], pss, AF.Copy, scale=1.0 / WS
                )
            else:
                nc.vector.tensor_scalar(
                    vt_sb[:, jt, :], pss, 1.0 / WS, None, OP.mult
                )

        # prelude — only what the first attention iterations need: the
        # ic=0 queries and the first few vt' tiles. The rest is deferred
        # into the attention window.
        for co in range(CI):
            emit_qm(co, 0)
        for jt in range(6):
            emit_vt(jt, act=(jt < 2))
        emit_apply(1, ["vec", "act", "pool", "vec"])

        rcols = []
        for ic in range(IC):
            ot_ps = [
                outp.tile([P, FD], f32, tag=f"ot{t}", name=f"ot{t}_{ic}")
                for t in range(FD // P)
            ]
            for u in range(JT // 2):
                pexp = pexpall[:, u, :, :]
                s2 = mxp.tile([P, 2, FD], f32, tag="mx", name=f"s{u}_{ic}")
                for t in range(2):
                    jt = 2 * u + t
                    for ep in range(CI // 2):
                        nc.tensor.matmul(
                            s2[:, t, :],
                            lhsT=xn[:, 2 * ep:2 * ep + 2, jt * P:(jt + 1) * P],
                            rhs=qm_sb[:, 2 * ep:2 * ep + 2, ic * FD:(ic + 1) * FD],
                            start=(ep == 0),
                            stop=(ep == CI // 2 - 1),
                            perf_mode=DR,
                        )
                # batched exp over both key tiles (one ScalarE op)
                nc.scalar.activation(pexp, s2, AF.Exp, scale=SCALE)
                for t in range(FD // P):
                    nc.tensor.matmul(
                        ot_ps[t],
                        lhsT=pexp[:, :, t * P:(t + 1) * P],
                        rhs=vt_sb[:, 2 * u:2 * u + 2, :],
                        start=(u == 0), stop=(u == JT // 2 - 1), perf_mode=DR,
                    )
                # software-pipelined production for upcoming iterations,
                # emitted after the attention block so S tiles win slots
                if ic == 0:
                    for jtn in (2 * u + 6, 2 * u + 7):
                        if jtn < JT:
                            emit_vt(jtn, act=(jtn % 4 == 3))
                    if u == 1:  # second query chunk, needed from ic=1 on
                        for co in range(CI):
                            emit_qm(co, 1)
                    if u == 2:
                        emit_apply(2, ["vec", "act", "pool", "vec"])
                    if u == 4:
                        emit_apply(3, ["vec", "act", "pool", "vec"])
            # batched softmax-denominator reduction off the hot loop
            dps = mxp.tile([P, 2, FD], f32, tag="mx", name=f"d_{ic}")
            d_ps = dps[0:1, 0, :]
            for u in range(JT // 2):
                nc.tensor.matmul(
                    d_ps, lhsT=ones2[:, :, 0:1], rhs=pexpall[:, u, :, :],
                    start=(u == 0), stop=(u == JT // 2 - 1), perf_mode=DR,
                )
            rrow = smallp.tile([1, FD], f32, tag=f"rrow{ic}", name=f"rrow{ic}")
            nc.vector.reciprocal(rrow, d_ps)  # 1/D, queries on free dim
            dcp = mxp.tile([P, 2, FD], f32, tag="mx", name=f"dc_{ic}")
            dc_ps = dcp[:, 0, 0:FD // P]
            for t in range(FD // P):
                nc.tensor.matmul(
                    dc_ps[:, t:t + 1],
                    lhsT=rrow[:, t * P:(t + 1) * P],
                    rhs=ones_1, start=True, stop=True,
                )
            rcol = smallp.tile([P, FD // P], f32, tag="rcol", bufs=2)
            nc.vector.tensor_copy(rcol, dc_ps)
            rcols.append(rcol)

            # ---- epilogue: out^T is already the projected output; scale by
            # 1/D (per-partition: queries on partitions) + residual add ----
            for t in range(FD // P):
                it = ic * (FD // P) + t
                ot = outst.tile([P, C], f32, tag="ot")
                nc.vector.tensor_scalar(
                    ot, ot_ps[t], rcol[:, t:t + 1], None, OP.mult
                )
                nc.gpsimd.tensor_add(ot, ot, xres_s[:, it, :])
                nc.sync.dma_start(out_t[:, it, :], ot)
        ph.close()

    nc.compile()  # bacc passes: wait legalization, event sems, nop fusion
    return nc


_NC = None


def _get_nc():
    global _NC
    if _NC is None:
        _NC = build_bass()
    return _NC


def _prep_core_inputs(x, gn_scale, gn_bias, qw, qb, kw, kb, vw, vb, pw, pb):
    """Build the 8 per-core input maps (host-side sharding / layout prep)."""
    f32 = np.float32
    f64 = np.float64

    def chunkP(a2d):  # [C, M] -> [128, C//128, M]
        Cdim, M = a2d.shape
        return np.ascontiguousarray(
            a2d.reshape(CI, P, M).transpose(1, 0, 2)
        )

    def colsP(v):  # [C] -> [128, CI]
        return np.ascontiguousarray(np.asarray(v, f32).reshape(CI, P).T)

    # host-side weight folds (f64 for exactness)
    qw64, kw64 = np.asarray(qw, f64), np.asarray(kw, f64)
    vw64, pw64 = np.asarray(vw, f64), np.asarray(pw, f64)
    M = qw64.T @ kw64                      # [c, d]: S = xn^T M xn
    Wpv = pw64 @ vw64                      # [o, c]: out = Wpv (xn P)
    u0 = kw64.T @ np.asarray(qb, f64)      # [d]: key-side bias term
    kc0 = pw64 @ np.asarray(vb, f64) + np.asarray(pb, f64)  # [o]

    g_red = np.zeros((P, 8), f32)
    for p in range(P):
        g_red[p, p // 16] = 1.0 / 16.0
    g_bc = np.zeros((8, P), f32)
    for p in range(P):
        g_bc[p // 16, p] = 1.0

    shared = {
        "mT": (chunkP(M.astype(f32)) * WS).astype(F8),
        "wpvT": (chunkP(Wpv.T.astype(f32)) * WS).astype(F8),
        "u0c": colsP(u0.astype(f32)),
        "kc0_bc": np.ascontiguousarray(
            np.broadcast_to(kc0.astype(f32), (P, C))
        ),
        "gns_t": colsP(gn_scale),
        "gnb_t": colsP(gn_bias),
        "g_red": g_red,
        "g_bc": g_bc,
    }

    xf = np.asarray(x, f32).reshape(B, C, N)
    in_maps = []
    for core in range(8):
        b, q = core // 4, core % 4
        # Roll pixels so this core's query quarter starts at pixel 0.
        # GN stats and attention-over-keys are permutation invariant, so
        # keys over rolled pixel order give identical results.
        xi = chunkP(np.roll(xf[b], -q * NQ, axis=1))  # [128, CI, N]
        xq = xf[b][:, q * NQ:(q + 1) * NQ]  # [C, NQ]
        xrT = np.ascontiguousarray(
            xq.T.reshape(IT, P, C).transpose(1, 0, 2)
        )  # [128, IT, C]
        in_maps.append({"x_img": xi, "x_resT": xrT, **shared})
    return in_maps


def _assemble(results):
    """results: list of 8 dicts with out_t [128, IT, C] -> [B, C, H, W]."""
    out = np.empty((B, C, N), np.float32)
    for core in range(8):
        b, q = core // 4, core % 4
        ot = np.asarray(results[core]["out_t"])  # [P, IT, C]
        # i_local = it*P + p ; out[b, :, q*NQ + i_local] = ot[p, it, :]
        blk = ot.transpose(1, 0, 2).reshape(NQ, C)  # [i_local, c]
        out[b, :, q * NQ:(q + 1) * NQ] = blk.T
    return out.reshape(B, C, H, W)


def kernel(**inputs):
    from concourse.bass_utils import run_bass_kernel_spmd

    nc = _get_nc()
    in_maps = _prep_core_inputs(**inputs)
    res = run_bass_kernel_spmd(nc, in_maps, core_ids=list(range(8)))
    return _assemble(res.results)


if __name__ == "__main__":
    nc = build_bass()
    print("built OK")


# revision 7
# speedup vs baseline: 1.1557x; 1.1557x over previous
"""AttnBlock (GroupNorm -> QKV 1x1 -> HxW self-attention -> proj -> residual)
as a Bass/Tile kernel on 8 TRN2 NeuronCores.

Sharding: data-parallel over batch B=2 and sequence-parallel over HW
quarters (4 cores per image, 1024 queries each). Each core redundantly
computes GroupNorm + full-image keys/values so there is no cross-core
communication. The host rolls the pixel axis per core so each core's
query quarter starts at pixel 0 (GN stats and attention-over-keys are
permutation invariant), letting all cores run one SPMD program.

Algebraic folds (host side) eliminate whole on-device phases:
  S   = q.k = (Wq xn + qb).(Wk xn + kb)
      = xn^T (Wq^T Wk) xn + (Wk^T qb).xn_j + [terms const over keys j,
        which softmax cancels]
    so with M = Wq^T Wk and u0 = Wk^T qb folded on host:
        qm = M^T xn_q + u0   (replaces the Q AND K projections)
        S  = qm^T xn         (keys are raw xn -- no K tensor at all)
  out = Wp (V P) + pb, V = Wv xn + vb, P = softmax rows
      = (Wp Wv) (xn P) + (pw vb + pb)
    so with Wpv = Wp Wv and kc0 = pw vb + pb folded on host:
        vt' = xn^T Wpv^T     (one value pass instead of V-proj + out-proj)
        out^T[i,:] = sum_j P[j,i] vt'[j,:]  -- the PV matmul emits the
        FINAL projected output directly in [query, channel] layout, so
        the epilogue is just a per-partition 1/D scale + residual add.
This removes ~1.3G MACs/core of matmul and ~100 PSUM->SBUF copy ops.

Precision: matmul operands in fp8e4 (E4M3) with DoubleRow perf mode;
fp32 PSUM accumulation. Folded weights are pre-scaled x256 on the host
(fp8 normal range); the 1/256 folds into existing psum->SBUF copies.
Softmax skips max-subtraction (logits in ~[-1.5, 1.5] by construction);
the softmax division is a per-partition scale in the epilogue.

Engine balance: exp runs in batched [128, 2x512] ops on ScalarE; the
GN-apply cast and the PSUM->SBUF copies are split across Vector/Scalar/
GpSimd; residual adds ride the otherwise idle GpSimd engine.
"""

import sys

sys.path.insert(0, "/opt/trn_rl_repo")

import numpy as np
import ml_dtypes

B, C, H, W = 2, 512, 64, 64
N = H * W            # 4096 pixels per image
NQ = N // 4          # 1024 queries per core
CI = C // 128        # 4 channel chunks of 128
NUM_GROUPS = 32
EPS = 1e-6
P = 128
FD = 512             # matmul moving free dim
JT = N // P          # 32 key tiles
IC = NQ // FD        # 2 query chunks of 512
IT = NQ // P         # 8 query tiles of 128
SCALE = float(C) ** -0.5
WS = 256.0           # host-side weight pre-scale (keeps fp8e4 in normal range)

F8 = ml_dtypes.float8_e4m3


def build_bass():
    import concourse.bass as bass
    import concourse.tile as tile
    import concourse.mybir as mybir
    from concourse import bacc
    from contextlib import ExitStack

    f32 = mybir.dt.float32
    f8 = mybir.dt.float8e4
    AF = mybir.ActivationFunctionType
    OP = mybir.AluOpType
    DR = mybir.MatmulPerfMode.DoubleRow

    nc = bacc.Bacc("TRN2")

    # ---------------- DRAM I/O ----------------
    x_img = nc.dram_tensor("x_img", [P, CI, N], f32, kind="ExternalInput")
    x_resT = nc.dram_tensor("x_resT", [P, IT, C], f32, kind="ExternalInput")
    mT = nc.dram_tensor("mT", [P, CI, C], f8, kind="ExternalInput")
    wpvT = nc.dram_tensor("wpvT", [P, CI, C], f8, kind="ExternalInput")
    u0c = nc.dram_tensor("u0c", [P, CI], f32, kind="ExternalInput")
    kc0_bc = nc.dram_tensor("kc0_bc", [P, C], f32, kind="ExternalInput")
    gns_t = nc.dram_tensor("gns_t", [P, CI], f32, kind="ExternalInput")
    gnb_t = nc.dram_tensor("gnb_t", [P, CI], f32, kind="ExternalInput")
    g_red = nc.dram_tensor("g_red", [P, 8], f32, kind="ExternalInput")
    g_bc = nc.dram_tensor("g_bc", [8, P], f32, kind="ExternalInput")
    out_t = nc.dram_tensor("out_t", [P, IT, C], f32, kind="ExternalOutput")

    with tile.TileContext(nc) as tc, ExitStack() as top:
        consts = top.enter_context(tc.tile_pool(name="consts", bufs=1))
        big = top.enter_context(tc.tile_pool(name="big", bufs=1))
        smallp = top.enter_context(tc.tile_pool(name="smallp", bufs=1))
        outst = top.enter_context(tc.tile_pool(name="outst", bufs=4))

        # x DMAs first — GroupNorm stats are the critical-path head, so x
        # must not queue behind the constant loads
        gnscope = ExitStack()
        xresid = gnscope.enter_context(tc.tile_pool(name="xresid", bufs=1))
        xr = [xresid.tile([P, N], f32, name=f"xr{ci}") for ci in range(CI)]
        for ci in range(CI):
            for h in range(2):
                nc.sync.dma_start(
                    xr[ci][:, h * 2048:(h + 1) * 2048],
                    x_img[:, ci, h * 2048:(h + 1) * 2048],
                )

        # ---- load constants ----
        # tiny GroupNorm constants first: the per-chunk reduce chain needs
        # them early, and they'd otherwise queue behind 9MB of x + weights
        gns_s = consts.tile([P, CI], f32)
        gnb_s = consts.tile([P, CI], f32)
        nc.sync.dma_start(gns_s, gns_t[:])
        nc.sync.dma_start(gnb_s, gnb_t[:])
        gr_s = consts.tile([P, 8], f32)
        gb_s = consts.tile([8, P], f32)
        nc.sync.dma_start(gr_s, g_red[:])
        nc.sync.dma_start(gb_s, g_bc[:])
        m_s = consts.tile([P, CI, C], f8)
        wpv_s = consts.tile([P, CI, C], f8)
        nc.sync.dma_start(m_s, mT[:])
        nc.sync.dma_start(wpv_s, wpvT[:])
        u0_s = consts.tile([P, CI], f32)
        nc.sync.dma_start(u0_s, u0c[:])
        kc0_s = consts.tile([P, C], f32)
        nc.sync.dma_start(kc0_s, kc0_bc[:])
        # padded to 16 so the DoubleRow pair-plane stride is 16B (%16 rule)
        ones2 = consts.tile([P, 2, 16], f8)
        nc.vector.memset(ones2, 1.0)
        ones_1 = consts.tile([1, 1], f32)
        nc.vector.memset(ones_1, 1.0)
        eps8 = consts.tile([8, 1], f32)
        nc.vector.memset(eps8, EPS)

        xres_s = big.tile([P, IT, C], f32)
        nc.sync.dma_start(xres_s, x_resT[:])

        # residual pre-adds (x + kc0 const row) on the otherwise-idle
        # GpSimd engine (SBUF-only); runs in the DMA-bound prelude
        for it in range(IT):
            nc.gpsimd.tensor_add(xres_s[:, it, :], xres_s[:, it, :], kc0_s)

        # big persistent tensors (fp8)
        xn = big.tile([P, CI, N], f8)            # normalized x (keys AND gn)
        vt_sb = big.tile([P, JT, C], f8)         # vt' = xn^T Wpv^T, [j, o]
        qm_sb = big.tile([P, CI, NQ], f8)        # qm = M^T xn_q + u0, [d, i]
        # exp(S), double-buffered per query-half, persisted so the
        # softmax-denominator reduction and the PV sweeps can run off the
        # hot loop without write-after-read stalls between halves
        pexpall = big.tile([P, IC, JT // 2, 2, FD], f8)

        # =============== Phase 1: GroupNorm ===============
        gnw = gnscope.enter_context(tc.tile_pool(name="gnw", bufs=1))
        gnps_scope = ExitStack()
        gnps = gnps_scope.enter_context(
            tc.tile_pool(name="gnps", bufs=1, space="PSUM")
        )

        mv2 = gnw.tile([P, 2 * CI], f32)  # per-channel (mean, m2) per chunk
        gps = gnps.tile([8, 2 * CI], f32, tag="g")
        gst = gnw.tile([8, 2 * CI], f32)
        bcps = gnps.tile([P, 2 * CI], f32, tag="bc")
        a_all = gnw.tile([P, CI], f32)
        b_all = gnw.tile([P, CI], f32)
        for ci in range(CI):
            bnst = gnw.tile([P, 8, 6], f32, tag="bnst", bufs=2)
            for s in range(8):
                nc.vector.bn_stats(
                    bnst[:, s, :], xr[ci][:, s * 512:(s + 1) * 512]
                )
            nc.vector.bn_aggr(mv2[:, 2 * ci:2 * ci + 2], bnst)
            # per-chunk reduce chain on ScalarE (idle during stats), so it
            # never steals VectorE from the stats stream; chunks 0..2 finish
            # under later chunks' DMA/stats and only chunk 3's chain is on
            # the critical path. Column math via activation scale/bias APs;
            # only the reciprocal must run on VectorE.
            mu = mv2[:, 2 * ci:2 * ci + 1]
            m2 = mv2[:, 2 * ci + 1:2 * ci + 2]
            sq = gnw.tile([P, 1], f32, tag="sq", bufs=2)
            nc.scalar.activation(sq, mu, AF.Copy, scale=mu)      # mean^2
            nc.scalar.activation(m2, sq, AF.Identity, bias=m2)   # 2nd moment
            nc.tensor.matmul(
                gps[:, 2 * ci:2 * ci + 2], lhsT=gr_s,
                rhs=mv2[:, 2 * ci:2 * ci + 2], start=True, stop=True,
            )
            gmu = gst[:, 2 * ci:2 * ci + 1]
            gm2 = gst[:, 2 * ci + 1:2 * ci + 2]
            nc.scalar.copy(gst[:, 2 * ci:2 * ci + 2], gps[:, 2 * ci:2 * ci + 2])
            gsq = gnw.tile([8, 1], f32, tag="gsq", bufs=2)
            nc.scalar.activation(gsq, gmu, AF.Copy, scale=gmu)   # gmean^2
            nc.scalar.activation(gm2, gsq, AF.Identity, scale=-1.0, bias=gm2)
            nc.scalar.activation(gm2, gm2, AF.Sqrt, bias=eps8)   # std
            nc.vector.reciprocal(gm2, gm2)                       # rstd
            nc.tensor.matmul(
                bcps[:, 2 * ci:2 * ci + 2], lhsT=gb_s,
                rhs=gst[:, 2 * ci:2 * ci + 2], start=True, stop=True,
            )
            a = a_all[:, ci:ci + 1]
            b = b_all[:, ci:ci + 1]
            chp = gnw.tile([P, 2], f32, tag="chp", bufs=2)
            nc.scalar.copy(chp, bcps[:, 2 * ci:2 * ci + 2])
            nc.scalar.activation(a, gns_s[:, ci:ci + 1], AF.Copy,
                                 scale=chp[:, 1:2])              # rstd*gns
            tmpc = gnw.tile([P, 1], f32, tag="tmpc", bufs=2)
            nc.scalar.activation(tmpc, chp[:, 0:1], AF.Copy, scale=a)
            nc.scalar.activation(b, tmpc, AF.Identity, scale=-1.0,
                                 bias=gnb_s[:, ci:ci + 1])

        def emit_apply(h, engines):
            # xn[:, ci, h-quarter] = a*x + b, split across engines
            for ci in range(CI):
                dst = xn[:, ci, h * 1024:(h + 1) * 1024]
                src = xr[ci][:, h * 1024:(h + 1) * 1024]
                eng = engines[ci % len(engines)]
                if eng == "act":
                    nc.scalar.activation(
                        dst, src, AF.Identity,
                        bias=b_all[:, ci:ci + 1], scale=a_all[:, ci:ci + 1],
                    )
                elif eng == "pool":
                    nc.gpsimd.tensor_scalar(
                        dst, src,
                        a_all[:, ci:ci + 1], b_all[:, ci:ci + 1],
                        OP.mult, OP.add,
                    )
                else:
                    nc.vector.tensor_scalar(
                        dst, src,
                        a_all[:, ci:ci + 1], b_all[:, ci:ci + 1],
                        OP.mult, OP.add,
                    )

        # quarter 0 first (it holds the queries + first key tiles); fan the
        # chunk casts across all three elementwise engines
        emit_apply(0, ["act", "vec", "pool", "act"])

        # psum pools for attention (8 banks): s-tiles 2x2, vt'/qm 2x1,
        # out^T accumulators 2x1; the GN psum banks must be gone first
        gnps_scope.close()
        ph = ExitStack()
        sxp = ph.enter_context(tc.tile_pool(name="sxp", bufs=2, space="PSUM"))
        vtp = ph.enter_context(tc.tile_pool(name="vtp", bufs=2, space="PSUM"))
        outp = ph.enter_context(tc.tile_pool(name="outp", bufs=1, space="PSUM"))

        def emit_qm(co, icq):
            # qm[d-block co, i-chunk icq] = M^T xn_q + u0
            ps = vtp.tile([P, FD], f32, tag="vt", name=f"qm{co}_{icq}")
            for ep in range(CI // 2):
                nc.tensor.matmul(
                    ps,
                    lhsT=m_s[:, 2 * ep:2 * ep + 2, co * P:(co + 1) * P],
                    rhs=xn[:, 2 * ep:2 * ep + 2, icq * FD:(icq + 1) * FD],
                    start=(ep == 0),
                    stop=(ep == CI // 2 - 1),
                    perf_mode=DR,
                )
            nc.scalar.activation(
                qm_sb[:, co, icq * FD:(icq + 1) * FD], ps, AF.Identity,
                bias=u0_s[:, co:co + 1], scale=1.0 / WS,
            )

        def emit_vt(jt, act=False):
            # vt'[j-tile jt, :] = xn^T Wpv^T
            ps = vtp.tile([P, FD], f32, tag="vt", name=f"vt{jt}")
            for ep in range(CI // 2):
                nc.tensor.matmul(
                    ps,
                    lhsT=xn[:, 2 * ep:2 * ep + 2, jt * P:(jt + 1) * P],
                    rhs=wpv_s[:, 2 * ep:2 * ep + 2, :],
                    start=(ep == 0),
                    stop=(ep == CI // 2 - 1),
                    perf_mode=DR,
                )
            if act:
                nc.scalar.activation(
                    vt_sb[:, jt, :], ps, AF.Copy, scale=1.0 / WS
                )
            else:
                nc.vector.tensor_scalar(
                    vt_sb[:, jt, :], ps, 1.0 / WS, None, OP.mult
                )

        def emit_s(ic, u):
            # S^T for key tiles (2u, 2u+1) x query chunk ic, then one
            # batched exp into the persisted pexp buffer
            s2 = sxp.tile([P, 2, FD], f32, tag="s", name=f"s{u}_{ic}")
            for t in range(2):
                jt = 2 * u + t
                for ep in range(CI // 2):
                    nc.tensor.matmul(
                        s2[:, t, :],
                        lhsT=xn[:, 2 * ep:2 * ep + 2, jt * P:(jt + 1) * P],
                        rhs=qm_sb[:, 2 * ep:2 * ep + 2, ic * FD:(ic + 1) * FD],
                        start=(ep == 0),
                        stop=(ep == CI // 2 - 1),
                        perf_mode=DR,
                    )
            nc.scalar.activation(pexpall[:, ic, u, :, :], s2, AF.Exp,
                                 scale=SCALE)

        def emit_pv(ic, u, t, dst, start, stop):
            nc.tensor.matmul(
                dst,
                lhsT=pexpall[:, ic, u, :, t * P:(t + 1) * P],
                rhs=vt_sb[:, 2 * u:2 * u + 2, :],
                start=start, stop=stop, perf_mode=DR,
            )

        def emit_epi(ic, t, src, rcol):
            it = ic * (FD // P) + t
            ot = outst.tile([P, C], f32, tag="ot")
            nc.vector.tensor_scalar(ot, src, rcol[:, t:t + 1], None, OP.mult)
            nc.gpsimd.tensor_add(ot, ot, xres_s[:, it, :])
            nc.sync.dma_start(out_t[:, it, :], ot)

        def emit_dchain(ic, d_ps, dc_ps, in_loop):
            if not in_loop:
                for u in range(JT // 2):
                    nc.tensor.matmul(
                        d_ps, lhsT=ones2[:, :, 0:1],
                        rhs=pexpall[:, ic, u, :, :],
                        start=(u == 0), stop=(u == JT // 2 - 1), perf_mode=DR,
                    )
            rrow = smallp.tile([1, FD], f32, tag=f"rrow{ic}", name=f"rrow{ic}")
            nc.vector.reciprocal(rrow, d_ps)  # 1/D, queries on free dim
            for t in range(FD // P):
                nc.tensor.matmul(
                    dc_ps[:, t:t + 1],
                    lhsT=rrow[:, t * P:(t + 1) * P],
                    rhs=ones_1, start=True, stop=True,
                )
            rcol = smallp.tile([P, FD // P], f32, tag="rcol", bufs=2)
            nc.vector.tensor_copy(rcol, dc_ps)
            return rcol

        # prelude — only what the first attention iterations need: the
        # ic=0 queries and the first few vt' tiles. The rest is deferred
        # into the attention window.
        for co in range(CI):
            emit_qm(co, 0)
        for jt in range(4):
            emit_vt(jt, act=(jt < 2))
        emit_apply(1, ["vec", "pool", "pool", "vec"])

        # ================= query chunk ic=0 =================
        # PV accumulates it-tiles {0,1} in-loop (outp); {2,3} run as a
        # post-loop sweep over the persisted pexp, overlapping ic=1's
        # S/exp stream. PE stream is skewed: PV(u-1) is emitted after
        # S(u) so PE never waits on the exp round-trip.
        ot01 = [
            outp.tile([P, FD], f32, tag=f"ot{t}", name=f"ot{t}_0")
            for t in range(2)
        ]
        for u in range(JT // 2):
            emit_s(0, u)
            if u > 0:
                for t in range(2):
                    emit_pv(0, u - 1, t, ot01[t], start=(u == 1), stop=False)
            # software-pipelined production for upcoming iterations
            for jtn in (2 * u + 4, 2 * u + 5):
                if jtn < JT:
                    emit_vt(jtn, act=(jtn % 4 == 3))
            if u == 1:  # second query chunk, needed from ic=1 on
                for co in range(CI):
                    emit_qm(co, 1)
            if u == 2:
                emit_apply(2, ["pool", "vec", "pool", "vec"])
            if u == 4:
                emit_apply(3, ["pool", "vec", "pool", "vec"])
        gnscope.close()
        for t in range(2):
            emit_pv(0, JT // 2 - 1, t, ot01[t], start=False, stop=True)

        # ic0 denominator chain (PE) — emitted before ic1's stream so the
        # epilogue scale can start; a couple of ic1 S units fill the PE
        # bubble while the reciprocal chain runs on DVE
        dps = sxp.tile([P, 2, FD], f32, tag="s", name="d_0")
        dcp = sxp.tile([P, 2, FD], f32, tag="s", name="dc_0")
        rcol0 = emit_dchain(0, dps[0:1, 0, :], dcp[:, 0, 0:FD // P], False)

        ot23 = [
            outp.tile([P, FD], f32, tag=f"ot{t}", name=f"ot{t}_0b")
            for t in range(2)
        ]
        emit_s(1, 0)
        emit_s(1, 1)
        for t in range(2):
            emit_epi(0, t, ot01[t], rcol0)
        # it {2,3} sweep: back-to-back accumulation over the persisted pexp
        for u in range(JT // 2):
            for t in range(2):
                emit_pv(0, u, 2 + t, ot23[t], start=(u == 0),
                        stop=(u == JT // 2 - 1))
        for t in range(2):
            emit_epi(0, 2 + t, ot23[t], rcol0)

        # ================= query chunk ic=1 =================
        # vt'/qm production is done, so the vtp banks serve as the extra
        # two PV accumulators: all four it-tiles accumulate in-loop.
        ot4 = [
            outp.tile([P, FD], f32, tag=f"ot{t}", name=f"ot{t}_1")
            for t in range(2)
        ] + [
            vtp.tile([P, FD], f32, tag="vt", name=f"otv{t}_1")
            for t in range(2)
        ]
        for u in range(JT // 2):
            if u >= 2:
                emit_s(1, u)
            if u > 0:
                for t in range(4):
                    emit_pv(1, u - 1, t, ot4[t], start=(u == 1), stop=False)
        for t in range(4):
            emit_pv(1, JT // 2 - 1, t, ot4[t], start=False, stop=True)
        dps1 = sxp.tile([P, 2, FD], f32, tag="s", name="d_1")
        dcp1 = sxp.tile([P, 2, FD], f32, tag="s", name="dc_1")
        rcol1 = emit_dchain(1, dps1[0:1, 0, :], dcp1[:, 0, 0:FD // P], False)
        for t in range(4):
            emit_epi(1, t, ot4[t], rcol1)
        ph.close()

    nc.compile()  # bacc passes: wait legalization, event sems, nop fusion
    return nc


_NC = None


def _get_nc():
    global _NC
    if _NC is None:
        _NC = build_bass()
    return _NC


def _prep_core_inputs(x, gn_scale, gn_bias, qw, qb, kw, kb, vw, vb, pw, pb):
    """Build the 8 per-core input maps (host-side sharding / layout prep)."""
    f32 = np.float32
    f64 = np.float64

    def chunkP(a2d):  # [C, M] -> [128, C//128, M]
        Cdim, M = a2d.shape
        return np.ascontiguousarray(
            a2d.reshape(CI, P, M).transpose(1, 0, 2)
        )

    def colsP(v):  # [C] -> [128, CI]
        return np.ascontiguousarray(np.asarray(v, f32).reshape(CI, P).T)

    # host-side weight folds (f64 for exactness)
    qw64, kw64 = np.asarray(qw, f64), np.asarray(kw, f64)
    vw64, pw64 = np.asarray(vw, f64), np.asarray(pw, f64)
    M = qw64.T @ kw64                      # [c, d]: S = xn^T M xn
    Wpv = pw64 @ vw64                      # [o, c]: out = Wpv (xn P)
    u0 = kw64.T @ np.asarray(qb, f64)      # [d]: key-side bias term
    kc0 = pw64 @ np.asarray(vb, f64) + np.asarray(pb, f64)  # [o]

    g_red = np.zeros((P, 8), f32)
    for p in range(P):
        g_red[p, p // 16] = 1.0 / 16.0
    g_bc = np.zeros((8, P), f32)
    for p in range(P):
        g_bc[p // 16, p] = 1.0

    shared = {
        "mT": (chunkP(M.astype(f32)) * WS).astype(F8),
        "wpvT": (chunkP(Wpv.T.astype(f32)) * WS).astype(F8),
        "u0c": colsP(u0.astype(f32)),
        "kc0_bc": np.ascontiguousarray(
            np.broadcast_to(kc0.astype(f32), (P, C))
        ),
        "gns_t": colsP(gn_scale),
        "gnb_t": colsP(gn_bias),
        "g_red": g_red,
        "g_bc": g_bc,
    }

    xf = np.asarray(x, f32).reshape(B, C, N)
    in_maps = []
    for core in range(8):
        b, q = core // 4, core % 4
        # Roll pixels so this core's query quarter starts at pixel 0.
        # GN stats and attention-over-keys are permutation invariant, so
        # keys over rolled pixel order give identical results.
        xi = chunkP(np.roll(xf[b], -q * NQ, axis=1))  # [128, CI, N]
        xq = xf[b][:, q * NQ:(q + 1) * NQ]  # [C, NQ]
        xrT = np.ascontiguousarray(
            xq.T.reshape(IT, P, C).transpose(1, 0, 2)
        )  # [128, IT, C]
        in_maps.append({"x_img": xi, "x_resT": xrT, **shared})
    return in_maps


def _assemble(results):
    """results: list of 8 dicts with out_t [128, IT, C] -> [B, C, H, W]."""
    out = np.empty((B, C, N), np.float32)
    for core in range(8):
        b, q = core // 4, core % 4
        ot = np.asarray(results[core]["out_t"])  # [P, IT, C]
        # i_local = it*P + p ; out[b, :, q*NQ + i_local] = ot[p, it, :]
        blk = ot.transpose(1, 0, 2).reshape(NQ, C)  # [i_local, c]
        out[b, :, q * NQ:(q + 1) * NQ] = blk.T
    return out.reshape(B, C, H, W)


def kernel(**inputs):
    from concourse.bass_utils import run_bass_kernel_spmd

    nc = _get_nc()
    in_maps = _prep_core_inputs(**inputs)
    res = run_bass_kernel_spmd(nc, in_maps, core_ids=list(range(8)))
    return _assemble(res.results)


if __name__ == "__main__":
    nc = build_bass()
    print("built OK")


# revision 11
# speedup vs baseline: 1.2631x; 1.0929x over previous
"""AttnBlock (GroupNorm -> QKV 1x1 -> HxW self-attention -> proj -> residual)
as a Bass/Tile kernel on 8 TRN2 NeuronCores.

Sharding: data-parallel over batch B=2 and sequence-parallel over HW
quarters (4 cores per image, 1024 queries each). Each core redundantly
computes GroupNorm + full-image keys/values so there is no cross-core
communication. The host rolls the pixel axis per core so each core's
query quarter starts at pixel 0 (GN stats and attention-over-keys are
permutation invariant), letting all cores run one SPMD program.

Algebraic folds (host side) eliminate whole on-device phases:
  S   = q.k = (Wq xn + qb).(Wk xn + kb)
      = xn^T (Wq^T Wk) xn + (Wk^T qb).xn_j + [terms const over keys j,
        which softmax cancels]
    so with M = Wq^T Wk and u0 = Wk^T qb folded on host:
        qm = M^T xn_q + u0   (replaces the Q AND K projections)
        S  = qm^T xn         (keys are raw xn -- no K tensor at all)
  out = Wp (V P) + pb, V = Wv xn + vb, P = softmax rows
      = (Wp Wv) (xn P) + (pw vb + pb)
    so with Wpv = Wp Wv and kc0 = pw vb + pb folded on host:
        vt' = xn^T Wpv^T     (one value pass instead of V-proj + out-proj)
        out^T[i,:] = sum_j P[j,i] vt'[j,:]  -- the PV matmul emits the
        FINAL projected output directly in [query, channel] layout, so
        the epilogue is just a per-partition 1/D scale + residual add.
This removes ~1.3G MACs/core of matmul and ~100 PSUM->SBUF copy ops.

Precision: matmul operands in fp8e4 (E4M3) with DoubleRow perf mode;
fp32 PSUM accumulation. Folded weights are pre-scaled x256 on the host
(fp8 normal range); the 1/256 folds into existing psum->SBUF copies.
Softmax skips max-subtraction (logits in ~[-1.5, 1.5] by construction);
the softmax division is a per-partition scale in the epilogue.

Engine balance: exp runs in batched [128, 2x512] ops on ScalarE; the
GN-apply cast and the PSUM->SBUF copies are split across Vector/Scalar/
GpSimd; residual adds ride the otherwise idle GpSimd engine.
"""

import sys

sys.path.insert(0, "/opt/trn_rl_repo")

import numpy as np
import ml_dtypes

B, C, H, W = 2, 512, 64, 64
N = H * W            # 4096 pixels per image
NQ = N // 4          # 1024 queries per core
CI = C // 128        # 4 channel chunks of 128
NUM_GROUPS = 32
EPS = 1e-6
P = 128
FD = 512             # matmul moving free dim
JT = N // P          # 32 key tiles
IC = NQ // FD        # 2 query chunks of 512
IT = NQ // P         # 8 query tiles of 128
SCALE = float(C) ** -0.5
WS = 256.0           # host-side weight pre-scale (keeps fp8e4 in normal range)

F8 = ml_dtypes.float8_e4m3


def build_bass():
    import concourse.bass as bass
    import concourse.tile as tile
    import concourse.mybir as mybir
    from concourse import bacc
    from contextlib import ExitStack

    f32 = mybir.dt.float32
    f8 = mybir.dt.float8e4
    AF = mybir.ActivationFunctionType
    OP = mybir.AluOpType
    DR = mybir.MatmulPerfMode.DoubleRow

    nc = bacc.Bacc("TRN2")

    # ---------------- DRAM I/O ----------------
    x_img = nc.dram_tensor("x_img", [P, CI, N], f32, kind="ExternalInput")
    x_resT = nc.dram_tensor("x_resT", [P, IT, C], f32, kind="ExternalInput")
    mT = nc.dram_tensor("mT", [P, CI, C], f8, kind="ExternalInput")
    wpvT = nc.dram_tensor("wpvT", [P, CI, C], f8, kind="ExternalInput")
    u0c = nc.dram_tensor("u0c", [P, CI], f32, kind="ExternalInput")
    kc0_bc = nc.dram_tensor("kc0_bc", [P, C], f32, kind="ExternalInput")
    gns_t = nc.dram_tensor("gns_t", [P, CI], f32, kind="ExternalInput")
    gnb_t = nc.dram_tensor("gnb_t", [P, CI], f32, kind="ExternalInput")
    g_red = nc.dram_tensor("g_red", [P, 8], f32, kind="ExternalInput")
    g_bc = nc.dram_tensor("g_bc", [8, P], f32, kind="ExternalInput")
    out_t = nc.dram_tensor("out_t", [P, IT, C], f32, kind="ExternalOutput")

    with tile.TileContext(nc) as tc, ExitStack() as top:
        consts = top.enter_context(tc.tile_pool(name="consts", bufs=1))
        big = top.enter_context(tc.tile_pool(name="big", bufs=1))
        smallp = top.enter_context(tc.tile_pool(name="smallp", bufs=1))
        outst = top.enter_context(tc.tile_pool(name="outst", bufs=4))

        # x DMAs first — GroupNorm stats are the critical-path head, so x
        # must not queue behind the constant loads
        gnscope = ExitStack()
        xresid = gnscope.enter_context(tc.tile_pool(name="xresid", bufs=1))
        xr = [xresid.tile([P, N], f32, name=f"xr{ci}") for ci in range(CI)]
        for ci in range(CI):
            for h in range(4):
                nc.sync.dma_start(
                    xr[ci][:, h * 1024:(h + 1) * 1024],
                    x_img[:, ci, h * 1024:(h + 1) * 1024],
                )

        # ---- load constants ----
        # tiny GroupNorm constants first: the per-chunk reduce chain needs
        # them early, and they'd otherwise queue behind 9MB of x + weights
        gns_s = consts.tile([P, CI], f32)
        gnb_s = consts.tile([P, CI], f32)
        nc.sync.dma_start(gns_s, gns_t[:])
        nc.sync.dma_start(gnb_s, gnb_t[:])
        gr_s = consts.tile([P, 8], f32)
        gb_s = consts.tile([8, P], f32)
        nc.sync.dma_start(gr_s, g_red[:])
        nc.sync.dma_start(gb_s, g_bc[:])
        m_s = consts.tile([P, CI, C], f8)
        wpv_s = consts.tile([P, CI, C], f8)
        nc.sync.dma_start(m_s, mT[:])
        nc.sync.dma_start(wpv_s, wpvT[:])
        u0_s = consts.tile([P, CI], f32)
        nc.sync.dma_start(u0_s, u0c[:])
        kc0_s = consts.tile([P, C], f32)
        nc.sync.dma_start(kc0_s, kc0_bc[:])
        # padded to 16 so the DoubleRow pair-plane stride is 16B (%16 rule)
        ones2 = consts.tile([P, 2, 16], f8)
        nc.vector.memset(ones2, 1.0)
        ones_1 = consts.tile([1, 1], f32)
        nc.vector.memset(ones_1, 1.0)
        eps8 = consts.tile([8, 1], f32)
        nc.vector.memset(eps8, EPS)

        xres_s = big.tile([P, IT, C], f32)
        nc.sync.dma_start(xres_s, x_resT[:])

        # residual pre-adds (x + kc0 const row) on the otherwise-idle
        # GpSimd engine (SBUF-only); runs in the DMA-bound prelude
        for it in range(IT):
            nc.gpsimd.tensor_add(xres_s[:, it, :], xres_s[:, it, :], kc0_s)

        # big persistent tensors (fp8)
        xn = big.tile([P, CI, N], f8)            # normalized x (keys AND gn)
        vt_sb = big.tile([P, JT, C], f8)         # vt' = xn^T Wpv^T, [j, o]
        qm_sb = big.tile([P, CI, NQ], f8)        # qm = M^T xn_q + u0, [d, i]
        # exp(S), double-buffered per query-half, persisted so the
        # softmax-denominator reduction and the PV sweeps can run off the
        # hot loop without write-after-read stalls between halves
        pexpall = big.tile([P, IC, JT // 2, 2, FD], f8)

        # =============== Phase 1: GroupNorm ===============
        gnw = gnscope.enter_context(tc.tile_pool(name="gnw", bufs=1))
        gnps_scope = ExitStack()
        gnps = gnps_scope.enter_context(
            tc.tile_pool(name="gnps", bufs=1, space="PSUM")
        )

        mv2 = gnw.tile([P, 2 * CI], f32)  # per-channel (mean, m2) per chunk
        gps = gnps.tile([8, 2 * CI], f32, tag="g")
        gst = gnw.tile([8, 2 * CI], f32)
        bcps = gnps.tile([P, 2 * CI], f32, tag="bc")
        a_all = gnw.tile([P, CI], f32)
        b_all = gnw.tile([P, CI], f32)
        for ci in range(CI):
            bnst = gnw.tile([P, 8, 6], f32, tag="bnst", bufs=2)
            for s in range(8):
                nc.vector.bn_stats(
                    bnst[:, s, :], xr[ci][:, s * 512:(s + 1) * 512]
                )
            nc.vector.bn_aggr(mv2[:, 2 * ci:2 * ci + 2], bnst)
            # per-chunk reduce chain on ScalarE (idle during stats), so it
            # never steals VectorE from the stats stream; chunks 0..2 finish
            # under later chunks' DMA/stats and only chunk 3's chain is on
            # the critical path. Column math via activation scale/bias APs;
            # only the reciprocal must run on VectorE.
            mu = mv2[:, 2 * ci:2 * ci + 1]
            m2 = mv2[:, 2 * ci + 1:2 * ci + 2]
            sq = gnw.tile([P, 1], f32, tag="sq", bufs=2)
            nc.scalar.activation(sq, mu, AF.Copy, scale=mu)      # mean^2
            nc.scalar.activation(m2, sq, AF.Identity, bias=m2)   # 2nd moment
            nc.tensor.matmul(
                gps[:, 2 * ci:2 * ci + 2], lhsT=gr_s,
                rhs=mv2[:, 2 * ci:2 * ci + 2], start=True, stop=True,
            )
            gmu = gst[:, 2 * ci:2 * ci + 1]
            gm2 = gst[:, 2 * ci + 1:2 * ci + 2]
            nc.scalar.copy(gst[:, 2 * ci:2 * ci + 2], gps[:, 2 * ci:2 * ci + 2])
            gsq = gnw.tile([8, 1], f32, tag="gsq", bufs=2)
            nc.scalar.activation(gsq, gmu, AF.Copy, scale=gmu)   # gmean^2
            nc.scalar.activation(gm2, gsq, AF.Identity, scale=-1.0, bias=gm2)
            nc.scalar.activation(gm2, gm2, AF.Sqrt, bias=eps8)   # std
            nc.vector.reciprocal(gm2, gm2)                       # rstd
            nc.tensor.matmul(
                bcps[:, 2 * ci:2 * ci + 2], lhsT=gb_s,
                rhs=gst[:, 2 * ci:2 * ci + 2], start=True, stop=True,
            )
            a = a_all[:, ci:ci + 1]
            b = b_all[:, ci:ci + 1]
            chp = gnw.tile([P, 2], f32, tag="chp", bufs=2)
            nc.scalar.copy(chp, bcps[:, 2 * ci:2 * ci + 2])
            nc.scalar.activation(a, gns_s[:, ci:ci + 1], AF.Copy,
                                 scale=chp[:, 1:2])              # rstd*gns
            tmpc = gnw.tile([P, 1], f32, tag="tmpc", bufs=2)
            nc.scalar.activation(tmpc, chp[:, 0:1], AF.Copy, scale=a)
            nc.scalar.activation(b, tmpc, AF.Identity, scale=-1.0,
                                 bias=gnb_s[:, ci:ci + 1])

        def emit_apply(h, engines):
            # xn[:, ci, h-quarter] = a*x + b, split across engines
            for ci in range(CI):
                dst = xn[:, ci, h * 1024:(h + 1) * 1024]
                src = xr[ci][:, h * 1024:(h + 1) * 1024]
                eng = engines[ci % len(engines)]
                if eng == "act":
                    nc.scalar.activation(
                        dst, src, AF.Identity,
                        bias=b_all[:, ci:ci + 1], scale=a_all[:, ci:ci + 1],
                    )
                elif eng == "pool":
                    nc.gpsimd.tensor_scalar(
                        dst, src,
                        a_all[:, ci:ci + 1], b_all[:, ci:ci + 1],
                        OP.mult, OP.add,
                    )
                else:
                    nc.vector.tensor_scalar(
                        dst, src,
                        a_all[:, ci:ci + 1], b_all[:, ci:ci + 1],
                        OP.mult, OP.add,
                    )

        # quarter 0 first (it holds the queries + first key tiles); fan the
        # chunk casts across all three elementwise engines. ci3's (a,b)
        # lands last, so it gets the otherwise-idle ScalarE.
        emit_apply(0, ["vec", "pool", "vec", "act"])

        # psum pools for attention (8 banks): s-tiles 2x2, vt'/qm 2x1,
        # out^T accumulators 2x1; the GN psum banks must be gone first
        gnps_scope.close()
        ph = ExitStack()
        sxp = ph.enter_context(tc.tile_pool(name="sxp", bufs=2, space="PSUM"))
        vtp = ph.enter_context(tc.tile_pool(name="vtp", bufs=2, space="PSUM"))
        outp = ph.enter_context(tc.tile_pool(name="outp", bufs=1, space="PSUM"))

        def emit_qm(co, icq, act=True):
            # qm[d-block co, i-chunk icq] = M^T xn_q + u0
            ps = vtp.tile([P, FD], f32, tag="vt", name=f"qm{co}_{icq}")
            for ep in range(CI // 2):
                nc.tensor.matmul(
                    ps,
                    lhsT=m_s[:, 2 * ep:2 * ep + 2, co * P:(co + 1) * P],
                    rhs=xn[:, 2 * ep:2 * ep + 2, icq * FD:(icq + 1) * FD],
                    start=(ep == 0),
                    stop=(ep == CI // 2 - 1),
                    perf_mode=DR,
                )
            if act:
                nc.scalar.activation(
                    qm_sb[:, co, icq * FD:(icq + 1) * FD], ps, AF.Identity,
                    bias=u0_s[:, co:co + 1], scale=1.0 / WS,
                )
            else:
                nc.vector.tensor_scalar(
                    qm_sb[:, co, icq * FD:(icq + 1) * FD], ps,
                    1.0 / WS, u0_s[:, co:co + 1], OP.mult, OP.add,
                )

        def emit_vt(jt, act=False):
            # vt'[j-tile jt, :] = xn^T Wpv^T
            ps = vtp.tile([P, FD], f32, tag="vt", name=f"vt{jt}")
            for ep in range(CI // 2):
                nc.tensor.matmul(
                    ps,
                    lhsT=xn[:, 2 * ep:2 * ep + 2, jt * P:(jt + 1) * P],
                    rhs=wpv_s[:, 2 * ep:2 * ep + 2, :],
                    start=(ep == 0),
                    stop=(ep == CI // 2 - 1),
                    perf_mode=DR,
                )
            if act:
                nc.scalar.activation(
                    vt_sb[:, jt, :], ps, AF.Copy, scale=1.0 / WS
                )
            else:
                nc.vector.tensor_scalar(
                    vt_sb[:, jt, :], ps, 1.0 / WS, None, OP.mult
                )

        def emit_s(ic, u):
            # S^T for key tiles (2u, 2u+1) x query chunk ic, then one
            # batched exp into the persisted pexp buffer
            s2 = sxp.tile([P, 2, FD], f32, tag="s", name=f"s{u}_{ic}")
            for t in range(2):
                jt = 2 * u + t
                for ep in range(CI // 2):
                    nc.tensor.matmul(
                        s2[:, t, :],
                        lhsT=xn[:, 2 * ep:2 * ep + 2, jt * P:(jt + 1) * P],
                        rhs=qm_sb[:, 2 * ep:2 * ep + 2, ic * FD:(ic + 1) * FD],
                        start=(ep == 0),
                        stop=(ep == CI // 2 - 1),
                        perf_mode=DR,
                    )
            nc.scalar.activation(pexpall[:, ic, u, :, :], s2, AF.Exp,
                                 scale=SCALE)

        def emit_pv(ic, u, t, dst, start, stop):
            nc.tensor.matmul(
                dst,
                lhsT=pexpall[:, ic, u, :, t * P:(t + 1) * P],
                rhs=vt_sb[:, 2 * u:2 * u + 2, :],
                start=start, stop=stop, perf_mode=DR,
            )

        def emit_epi(ic, t, src, rcol):
            it = ic * (FD // P) + t
            ot = outst.tile([P, C], f32, tag="ot")
            nc.vector.tensor_scalar(ot, src, rcol[:, t:t + 1], None, OP.mult)
            nc.gpsimd.tensor_add(ot, ot, xres_s[:, it, :])
            nc.sync.dma_start(out_t[:, it, :], ot)

        def emit_dchain(ic, d_ps, dc_ps, in_loop):
            if not in_loop:
                for u in range(JT // 2):
                    nc.tensor.matmul(
                        d_ps, lhsT=ones2[:, :, 0:1],
                        rhs=pexpall[:, ic, u, :, :],
                        start=(u == 0), stop=(u == JT // 2 - 1), perf_mode=DR,
                    )
            rrow = smallp.tile([1, FD], f32, tag=f"rrow{ic}", name=f"rrow{ic}")
            nc.vector.reciprocal(rrow, d_ps)  # 1/D, queries on free dim
            for t in range(FD // P):
                nc.tensor.matmul(
                    dc_ps[:, t:t + 1],
                    lhsT=rrow[:, t * P:(t + 1) * P],
                    rhs=ones_1, start=True, stop=True,
                )
            rcol = smallp.tile([P, FD // P], f32, tag="rcol", bufs=2)
            nc.vector.tensor_copy(rcol, dc_ps)
            return rcol

        # prelude — only what the first attention iterations need: the
        # ic=0 queries and the first few vt' tiles. The rest is deferred
        # into the attention window. Copies ride the idle ScalarE (qm) and
        # VectorE (vt) so neither queue convoys behind the other.
        for co in range(CI):
            emit_qm(co, 0, act=True)
        for jt in range(4):
            emit_vt(jt, act=False)
        emit_apply(1, ["vec", "pool", "pool", "vec"])

        # ================= query chunk ic=0 =================
        # PV accumulates it-tiles {0,1} in-loop (outp); {2,3} run as a
        # sweep over the persisted pexp, interleaved into ic=1's u-loop
        # (4 matmuls per step) on the vtp banks once vt' production ends.
        # PE stream is skewed: PV(u-1) is emitted after S(u) so PE never
        # waits on the exp round-trip.
        ot01_0 = [
            outp.tile([P, FD], f32, tag=f"ot{t}", name=f"ot{t}_0")
            for t in range(2)
        ]
        for u in range(JT // 2):
            emit_s(0, u)
            if u > 0:
                for t in range(2):
                    emit_pv(0, u - 1, t, ot01_0[t], start=(u == 1),
                            stop=False)
            # software-pipelined production for upcoming iterations
            for jtn in (2 * u + 4, 2 * u + 5):
                if jtn < JT:
                    emit_vt(jtn, act=(jtn % 8 == 7))
            if u == 1:  # second query chunk, needed from ic=1 on
                for co in range(CI):
                    emit_qm(co, 1, act=False)
            if u == 2:
                emit_apply(2, ["pool", "vec", "pool", "vec"])
            if u == 4:
                emit_apply(3, ["pool", "vec", "pool", "vec"])
        gnscope.close()
        for t in range(2):
            emit_pv(0, JT // 2 - 1, t, ot01_0[t], start=False, stop=True)

        # ---- boundary: first ic1 S units keep PE/ScalarE busy while the
        # ic0 denominator chain and epilogue {0,1} run ----
        emit_s(1, 0)
        emit_s(1, 1)
        dps = sxp.tile([P, 2, FD], f32, tag="s", name="d_0")
        dcp = sxp.tile([P, 2, FD], f32, tag="s", name="dc_0")
        rcol0 = emit_dchain(0, dps[0:1, 0, :], dcp[:, 0, 0:FD // P], False)
        for t in range(2):
            emit_epi(0, t, ot01_0[t], rcol0)

        # ================= query chunk ic=1 =================
        ot01_1 = [
            outp.tile([P, FD], f32, tag=f"ot{t}", name=f"ot{t}_1")
            for t in range(2)
        ]
        ot23_0 = [
            vtp.tile([P, FD], f32, tag="vt", name=f"otv{t}_0")
            for t in range(2)
        ]
        ot23_1 = None
        for u in range(2, JT // 2):
            emit_s(1, u)
            if u == 2:
                for t in range(2):
                    emit_pv(1, 0, t, ot01_1[t], start=True, stop=False)
            for t in range(2):
                emit_pv(1, u - 1, t, ot01_1[t], start=False, stop=False)
            if 2 <= u <= 9:
                # ic0 it{2,3} sweep: 4 matmuls per step
                for su in (2 * (u - 2), 2 * (u - 2) + 1):
                    for t in range(2):
                        emit_pv(0, su, 2 + t, ot23_0[t], start=(su == 0),
                                stop=(su == JT // 2 - 1))
            if u == 9:
                for t in range(2):
                    emit_epi(0, 2 + t, ot23_0[t], rcol0)
            if u == 10:
                ot23_1 = [
                    vtp.tile([P, FD], f32, tag="vt", name=f"otv{t}_1")
                    for t in range(2)
                ]
            if u >= 10:
                # ic1 it{2,3} catch-up sweep on the freed vtp banks
                for su in (2 * (u - 10), 2 * (u - 10) + 1):
                    for t in range(2):
                        emit_pv(1, su, 2 + t, ot23_1[t], start=(su == 0),
                                stop=False)
        for t in range(2):
            emit_pv(1, JT // 2 - 1, t, ot01_1[t], start=False, stop=True)
        for su in range(12, JT // 2):
            for t in range(2):
                emit_pv(1, su, 2 + t, ot23_1[t], start=False,
                        stop=(su == JT // 2 - 1))
        dps1 = sxp.tile([P, 2, FD], f32, tag="s", name="d_1")
        dcp1 = sxp.tile([P, 2, FD], f32, tag="s", name="dc_1")
        rcol1 = emit_dchain(1, dps1[0:1, 0, :], dcp1[:, 0, 0:FD // P], False)
        for t in range(2):
            emit_epi(1, t, ot01_1[t], rcol1)
        for t in range(2):
            emit_epi(1, 2 + t, ot23_1[t], rcol1)
        ph.close()

    nc.compile()  # bacc passes: wait legalization, event sems, nop fusion
    return nc


_NC = None


def _get_nc():
    global _NC
    if _NC is None:
        _NC = build_bass()
    return _NC


def _prep_core_inputs(x, gn_scale, gn_bias, qw, qb, kw, kb, vw, vb, pw, pb):
    """Build the 8 per-core input maps (host-side sharding / layout prep)."""
    f32 = np.float32
    f64 = np.float64

    def chunkP(a2d):  # [C, M] -> [128, C//128, M]
        Cdim, M = a2d.shape
        return np.ascontiguousarray(
            a2d.reshape(CI, P, M).transpose(1, 0, 2)
        )

    def colsP(v):  # [C] -> [128, CI]
        return np.ascontiguousarray(np.asarray(v, f32).reshape(CI, P).T)

    # host-side weight folds (f64 for exactness)
    qw64, kw64 = np.asarray(qw, f64), np.asarray(kw, f64)
    vw64, pw64 = np.asarray(vw, f64), np.asarray(pw, f64)
    M = qw64.T @ kw64                      # [c, d]: S = xn^T M xn
    Wpv = pw64 @ vw64                      # [o, c]: out = Wpv (xn P)
    u0 = kw64.T @ np.asarray(qb, f64)      # [d]: key-side bias term
    kc0 = pw64 @ np.asarray(vb, f64) + np.asarray(pb, f64)  # [o]

    g_red = np.zeros((P, 8), f32)
    for p in range(P):
        g_red[p, p // 16] = 1.0 / 16.0
    g_bc = np.zeros((8, P), f32)
    for p in range(P):
        g_bc[p // 16, p] = 1.0

    shared = {
        "mT": (chunkP(M.astype(f32)) * WS).astype(F8),
        "wpvT": (chunkP(Wpv.T.astype(f32)) * WS).astype(F8),
        "u0c": colsP(u0.astype(f32)),
        "kc0_bc": np.ascontiguousarray(
            np.broadcast_to(kc0.astype(f32), (P, C))
        ),
        "gns_t": colsP(gn_scale),
        "gnb_t": colsP(gn_bias),
        "g_red": g_red,
        "g_bc": g_bc,
    }

    xf = np.asarray(x, f32).reshape(B, C, N)
    in_maps = []
    for core in range(8):
        b, q = core // 4, core % 4
        # Roll pixels so this core's query quarter starts at pixel 0.
        # GN stats and attention-over-keys are permutation invariant, so
        # keys over rolled pixel order give identical results.
        xi = chunkP(np.roll(xf[b], -q * NQ, axis=1))  # [128, CI, N]
        xq = xf[b][:, q * NQ:(q + 1) * NQ]  # [C, NQ]
        xrT = np.ascontiguousarray(
            xq.T.reshape(IT, P, C).transpose(1, 0, 2)
        )  # [128, IT, C]
        in_maps.append({"x_img": xi, "x_resT": xrT, **shared})
    return in_maps


def _assemble(results):
    """results: list of 8 dicts with out_t [128, IT, C] -> [B, C, H, W]."""
    out = np.empty((B, C, N), np.float32)
    for core in range(8):
        b, q = core // 4, core % 4
        ot = np.asarray(results[core]["out_t"])  # [P, IT, C]
        # i_local = it*P + p ; out[b, :, q*NQ + i_local] = ot[p, it, :]
        blk = ot.transpose(1, 0, 2).reshape(NQ, C)  # [i_local, c]
        out[b, :, q * NQ:(q + 1) * NQ] = blk.T
    return out.reshape(B, C, H, W)


def kernel(**inputs):
    from concourse.bass_utils import run_bass_kernel_spmd

    nc = _get_nc()
    in_maps = _prep_core_inputs(**inputs)
    res = run_bass_kernel_spmd(nc, in_maps, core_ids=list(range(8)))
    return _assemble(res.results)


if __name__ == "__main__":
    nc = build_bass()
    print("built OK")


# revision 15
# speedup vs baseline: 1.2829x; 1.0157x over previous
"""AttnBlock (GroupNorm -> QKV 1x1 -> HxW self-attention -> proj -> residual)
as a Bass/Tile kernel on 8 TRN2 NeuronCores.

Sharding: data-parallel over batch B=2 and sequence-parallel over HW
quarters (4 cores per image, 1024 queries each). Each core redundantly
computes GroupNorm + full-image keys/values so there is no cross-core
communication. The host rolls the pixel axis per core so each core's
query quarter starts at pixel 0 (GN stats and attention-over-keys are
permutation invariant), letting all cores run one SPMD program.

Algebraic folds (host side) eliminate whole on-device phases:
  S   = q.k = (Wq xn + qb).(Wk xn + kb)
      = xn^T (Wq^T Wk) xn + (Wk^T qb).xn_j + [terms const over keys j,
        which softmax cancels]
    so with M = Wq^T Wk and u0 = Wk^T qb folded on host:
        qm = M^T xn_q + u0   (replaces the Q AND K projections)
        S  = qm^T xn         (keys are raw xn -- no K tensor at all)
  out = Wp (V P) + pb, V = Wv xn + vb, P = softmax rows
      = (Wp Wv) (xn P) + (pw vb + pb)
    so with Wpv = Wp Wv and kc0 = pw vb + pb folded on host:
        vt' = xn^T Wpv^T     (one value pass instead of V-proj + out-proj)
        out^T[i,:] = sum_j P[j,i] vt'[j,:]  -- the PV matmul emits the
        FINAL projected output directly in [query, channel] layout, so
        the epilogue is just a per-partition 1/D scale + residual add.
This removes ~1.3G MACs/core of matmul and ~100 PSUM->SBUF copy ops.

Precision: matmul operands in fp8e4 (E4M3) with DoubleRow perf mode;
fp32 PSUM accumulation. Folded weights are pre-scaled x256 on the host
(fp8 normal range); the 1/256 folds into existing psum->SBUF copies.
Softmax skips max-subtraction (logits in ~[-1.5, 1.5] by construction);
the softmax division is a per-partition scale in the epilogue.

Engine balance: exp runs in batched [128, 2x512] ops on ScalarE; the
GN-apply cast and the PSUM->SBUF copies are split across Vector/Scalar/
GpSimd; residual adds ride the otherwise idle GpSimd engine.
"""

import sys

sys.path.insert(0, "/opt/trn_rl_repo")

import numpy as np
import ml_dtypes

B, C, H, W = 2, 512, 64, 64
N = H * W            # 4096 pixels per image
NQ = N // 4          # 1024 queries per core
CI = C // 128        # 4 channel chunks of 128
NUM_GROUPS = 32
EPS = 1e-6
P = 128
FD = 512             # matmul moving free dim
JT = N // P          # 32 key tiles
IC = NQ // FD        # 2 query chunks of 512
IT = NQ // P         # 8 query tiles of 128
SCALE = float(C) ** -0.5
WS = 256.0           # host-side weight pre-scale (keeps fp8e4 in normal range)

F8 = ml_dtypes.float8_e4m3


def build_bass():
    import concourse.bass as bass
    import concourse.tile as tile
    import concourse.mybir as mybir
    from concourse import bacc
    from contextlib import ExitStack

    f32 = mybir.dt.float32
    f8 = mybir.dt.float8e4
    AF = mybir.ActivationFunctionType
    OP = mybir.AluOpType
    DR = mybir.MatmulPerfMode.DoubleRow

    nc = bacc.Bacc("TRN2")

    # ---------------- DRAM I/O ----------------
    x_img = nc.dram_tensor("x_img", [P, CI, N], f32, kind="ExternalInput")
    x_resT = nc.dram_tensor("x_resT", [P, IT, C], f32, kind="ExternalInput")
    mT = nc.dram_tensor("mT", [P, CI, C], f8, kind="ExternalInput")
    wpvT = nc.dram_tensor("wpvT", [P, CI, C], f8, kind="ExternalInput")
    u0c = nc.dram_tensor("u0c", [P, CI], f32, kind="ExternalInput")
    kc0_bc = nc.dram_tensor("kc0_bc", [P, C], f32, kind="ExternalInput")
    gns_t = nc.dram_tensor("gns_t", [P, CI], f32, kind="ExternalInput")
    gnb_t = nc.dram_tensor("gnb_t", [P, CI], f32, kind="ExternalInput")
    g_red = nc.dram_tensor("g_red", [P, 8], f32, kind="ExternalInput")
    g_bc = nc.dram_tensor("g_bc", [8, P], f32, kind="ExternalInput")
    out_t = nc.dram_tensor("out_t", [P, IT, C], f32, kind="ExternalOutput")

    with tile.TileContext(nc) as tc, ExitStack() as top:
        consts = top.enter_context(tc.tile_pool(name="consts", bufs=1))
        big = top.enter_context(tc.tile_pool(name="big", bufs=1))
        smallp = top.enter_context(tc.tile_pool(name="smallp", bufs=1))
        outst = top.enter_context(tc.tile_pool(name="outst", bufs=4))

        # x DMAs first — GroupNorm stats are the critical-path head, so x
        # must not queue behind the constant loads
        gnscope = ExitStack()
        xresid = gnscope.enter_context(tc.tile_pool(name="xresid", bufs=1))
        xr = [xresid.tile([P, N], f32, name=f"xr{ci}") for ci in range(CI)]
        for ci in range(CI):
            for h in range(4):
                nc.sync.dma_start(
                    xr[ci][:, h * 1024:(h + 1) * 1024],
                    x_img[:, ci, h * 1024:(h + 1) * 1024],
                )

        # ---- load constants ----
        # tiny GroupNorm constants first: the per-chunk reduce chain needs
        # them early, and they'd otherwise queue behind 9MB of x + weights
        gns_s = consts.tile([P, CI], f32)
        gnb_s = consts.tile([P, CI], f32)
        nc.sync.dma_start(gns_s, gns_t[:])
        nc.sync.dma_start(gnb_s, gnb_t[:])
        gr_s = consts.tile([P, 8], f32)
        gb_s = consts.tile([8, P], f32)
        nc.sync.dma_start(gr_s, g_red[:])
        nc.sync.dma_start(gb_s, g_bc[:])
        m_s = consts.tile([P, CI, C], f8)
        wpv_s = consts.tile([P, CI, C], f8)
        nc.sync.dma_start(m_s, mT[:])
        nc.sync.dma_start(wpv_s, wpvT[:])
        u0_s = consts.tile([P, CI], f32)
        nc.sync.dma_start(u0_s, u0c[:])
        kc0_s = consts.tile([P, C], f32)
        nc.sync.dma_start(kc0_s, kc0_bc[:])
        # padded to 16 so the DoubleRow pair-plane stride is 16B (%16 rule)
        # (memsets on GpSimd: VectorE's in-order queue must stay clear for
        # the GroupNorm stats stream)
        ones2 = consts.tile([P, 2, 16], f8)
        nc.gpsimd.memset(ones2, 1.0)
        ones_1 = consts.tile([1, 1], f32)
        nc.gpsimd.memset(ones_1, 1.0)
        eps8 = consts.tile([8, 1], f32)
        nc.gpsimd.memset(eps8, EPS)

        xres_s = big.tile([P, IT, C], f32)
        nc.sync.dma_start(xres_s, x_resT[:])

        # residual pre-adds (x + kc0 const row) on the otherwise-idle
        # GpSimd engine (SBUF-only); runs in the DMA-bound prelude
        for it in range(IT):
            nc.gpsimd.tensor_add(xres_s[:, it, :], xres_s[:, it, :], kc0_s)

        # big persistent tensors (fp8)
        xn = big.tile([P, CI, N], f8)            # normalized x (keys AND gn)
        vt_sb = big.tile([P, JT, C], f8)         # vt' = xn^T Wpv^T, [j, o]
        qm_sb = big.tile([P, CI, NQ], f8)        # qm = M^T xn_q + u0, [d, i]
        # exp(S), double-buffered per query-half, persisted so the
        # softmax-denominator reduction and the PV sweeps can run off the
        # hot loop without write-after-read stalls between halves
        pexpall = big.tile([P, IC, JT // 2, 2, FD], f8)

        # =============== Phase 1: GroupNorm ===============
        gnw = gnscope.enter_context(tc.tile_pool(name="gnw", bufs=1))
        gnps_scope = ExitStack()
        gnps = gnps_scope.enter_context(
            tc.tile_pool(name="gnps", bufs=1, space="PSUM")
        )

        mv2 = gnw.tile([P, 2 * CI], f32)  # per-channel (mean, m2) per chunk
        gps = gnps.tile([8, 2 * CI], f32, tag="g")
        gst = gnw.tile([8, 2 * CI], f32)
        bcps = gnps.tile([P, 2 * CI], f32, tag="bc")
        a_all = gnw.tile([P, CI], f32)
        b_all = gnw.tile([P, CI], f32)
        # Pass 1 — the DVE queue carries ONLY stats (bn_stats/bn_aggr), so
        # it tracks the x DMA stream with no convoying; the ScalarE chain
        # up to sqrt(var) runs per-chunk in parallel. The reciprocals (DVE)
        # are deferred to pass 2 so they never block later chunks' stats.
        for ci in range(CI):
            bnst = gnw.tile([P, 8, 6], f32, tag="bnst", bufs=2)
            for s in range(8):
                nc.vector.bn_stats(
                    bnst[:, s, :], xr[ci][:, s * 512:(s + 1) * 512]
                )
            nc.vector.bn_aggr(mv2[:, 2 * ci:2 * ci + 2], bnst)
            mu = mv2[:, 2 * ci:2 * ci + 1]
            m2 = mv2[:, 2 * ci + 1:2 * ci + 2]
            sq = gnw.tile([P, 1], f32, tag="sq", bufs=2)
            nc.scalar.activation(sq, mu, AF.Copy, scale=mu)      # mean^2
            nc.scalar.activation(m2, sq, AF.Identity, bias=m2)   # 2nd moment
            nc.tensor.matmul(
                gps[:, 2 * ci:2 * ci + 2], lhsT=gr_s,
                rhs=mv2[:, 2 * ci:2 * ci + 2], start=True, stop=True,
            )
            gmu = gst[:, 2 * ci:2 * ci + 1]
            gm2 = gst[:, 2 * ci + 1:2 * ci + 2]
            nc.scalar.copy(gst[:, 2 * ci:2 * ci + 2], gps[:, 2 * ci:2 * ci + 2])
            gsq = gnw.tile([8, 1], f32, tag="gsq", bufs=2)
            nc.scalar.activation(gsq, gmu, AF.Copy, scale=gmu)   # gmean^2
            nc.scalar.activation(gm2, gsq, AF.Identity, scale=-1.0, bias=gm2)
            nc.scalar.activation(gm2, gm2, AF.Sqrt, bias=eps8)   # std
        # Pass 2 — reciprocals and the (a, b) affine coefficients
        for ci in range(CI):
            gm2 = gst[:, 2 * ci + 1:2 * ci + 2]
            nc.vector.reciprocal(gm2, gm2)                       # rstd
            nc.tensor.matmul(
                bcps[:, 2 * ci:2 * ci + 2], lhsT=gb_s,
                rhs=gst[:, 2 * ci:2 * ci + 2], start=True, stop=True,
            )
            a = a_all[:, ci:ci + 1]
            b = b_all[:, ci:ci + 1]
            chp = gnw.tile([P, 2], f32, tag="chp", bufs=2)
            nc.scalar.copy(chp, bcps[:, 2 * ci:2 * ci + 2])
            nc.scalar.activation(a, gns_s[:, ci:ci + 1], AF.Copy,
                                 scale=chp[:, 1:2])              # rstd*gns
            tmpc = gnw.tile([P, 1], f32, tag="tmpc", bufs=2)
            nc.scalar.activation(tmpc, chp[:, 0:1], AF.Copy, scale=a)
            nc.scalar.activation(b, tmpc, AF.Identity, scale=-1.0,
                                 bias=gnb_s[:, ci:ci + 1])

        def emit_apply(h, engines):
            # xn[:, ci, h-quarter] = a*x + b, split across engines
            for ci in range(CI):
                dst = xn[:, ci, h * 1024:(h + 1) * 1024]
                src = xr[ci][:, h * 1024:(h + 1) * 1024]
                eng = engines[ci % len(engines)]
                if eng == "act":
                    nc.scalar.activation(
                        dst, src, AF.Identity,
                        bias=b_all[:, ci:ci + 1], scale=a_all[:, ci:ci + 1],
                    )
                elif eng == "pool":
                    nc.gpsimd.tensor_scalar(
                        dst, src,
                        a_all[:, ci:ci + 1], b_all[:, ci:ci + 1],
                        OP.mult, OP.add,
                    )
                else:
                    nc.vector.tensor_scalar(
                        dst, src,
                        a_all[:, ci:ci + 1], b_all[:, ci:ci + 1],
                        OP.mult, OP.add,
                    )

        # quarter 0 first (it holds the queries + first key tiles); fan the
        # chunk casts across all three elementwise engines. ci3's (a,b)
        # lands last, so it gets the otherwise-idle ScalarE.
        emit_apply(0, ["vec", "pool", "vec", "act"])

        # psum pools for attention (8 banks): s-tiles 2x2, vt'/qm 2x1,
        # out^T accumulators 2x1; the GN psum banks must be gone first
        gnps_scope.close()
        ph = ExitStack()
        sxp = ph.enter_context(tc.tile_pool(name="sxp", bufs=2, space="PSUM"))
        vtp = ph.enter_context(tc.tile_pool(name="vtp", bufs=2, space="PSUM"))
        outp = ph.enter_context(tc.tile_pool(name="outp", bufs=1, space="PSUM"))

        def emit_qm(co, icq, act=True):
            # qm[d-block co, i-chunk icq] = M^T xn_q + u0
            ps = vtp.tile([P, FD], f32, tag="vt", name=f"qm{co}_{icq}")
            for ep in range(CI // 2):
                nc.tensor.matmul(
                    ps,
                    lhsT=m_s[:, 2 * ep:2 * ep + 2, co * P:(co + 1) * P],
                    rhs=xn[:, 2 * ep:2 * ep + 2, icq * FD:(icq + 1) * FD],
                    start=(ep == 0),
                    stop=(ep == CI // 2 - 1),
                    perf_mode=DR,
                )
            if act:
                nc.scalar.activation(
                    qm_sb[:, co, icq * FD:(icq + 1) * FD], ps, AF.Identity,
                    bias=u0_s[:, co:co + 1], scale=1.0 / WS,
                )
            else:
                nc.vector.tensor_scalar(
                    qm_sb[:, co, icq * FD:(icq + 1) * FD], ps,
                    1.0 / WS, u0_s[:, co:co + 1], OP.mult, OP.add,
                )

        def emit_vt(jt, act=False):
            # vt'[j-tile jt, :] = xn^T Wpv^T
            ps = vtp.tile([P, FD], f32, tag="vt", name=f"vt{jt}")
            for ep in range(CI // 2):
                nc.tensor.matmul(
                    ps,
                    lhsT=xn[:, 2 * ep:2 * ep + 2, jt * P:(jt + 1) * P],
                    rhs=wpv_s[:, 2 * ep:2 * ep + 2, :],
                    start=(ep == 0),
                    stop=(ep == CI // 2 - 1),
                    perf_mode=DR,
                )
            if act:
                nc.scalar.activation(
                    vt_sb[:, jt, :], ps, AF.Copy, scale=1.0 / WS
                )
            else:
                nc.vector.tensor_scalar(
                    vt_sb[:, jt, :], ps, 1.0 / WS, None, OP.mult
                )

        def emit_s(ic, u):
            # S^T for key tiles (2u, 2u+1) x query chunk ic, then one
            # batched exp into the persisted pexp buffer
            s2 = sxp.tile([P, 2, FD], f32, tag="s", name=f"s{u}_{ic}")
            for t in range(2):
                jt = 2 * u + t
                for ep in range(CI // 2):
                    nc.tensor.matmul(
                        s2[:, t, :],
                        lhsT=xn[:, 2 * ep:2 * ep + 2, jt * P:(jt + 1) * P],
                        rhs=qm_sb[:, 2 * ep:2 * ep + 2, ic * FD:(ic + 1) * FD],
                        start=(ep == 0),
                        stop=(ep == CI // 2 - 1),
                        perf_mode=DR,
                    )
            nc.scalar.activation(pexpall[:, ic, u, :, :], s2, AF.Exp,
                                 scale=SCALE)

        def emit_pv(ic, u, t, dst, start, stop):
            nc.tensor.matmul(
                dst,
                lhsT=pexpall[:, ic, u, :, t * P:(t + 1) * P],
                rhs=vt_sb[:, 2 * u:2 * u + 2, :],
                start=start, stop=stop, perf_mode=DR,
            )

        def emit_epi(ic, t, src, rcol):
            it = ic * (FD // P) + t
            ot = outst.tile([P, C], f32, tag="ot")
            nc.vector.tensor_scalar(ot, src, rcol[:, t:t + 1], None, OP.mult)
            nc.gpsimd.tensor_add(ot, ot, xres_s[:, it, :])
            nc.sync.dma_start(out_t[:, it, :], ot)

        def emit_dchain(ic, d_ps, dc_ps, in_loop):
            if not in_loop:
                for u in range(JT // 2):
                    nc.tensor.matmul(
                        d_ps, lhsT=ones2[:, :, 0:1],
                        rhs=pexpall[:, ic, u, :, :],
                        start=(u == 0), stop=(u == JT // 2 - 1), perf_mode=DR,
                    )
            rrow = smallp.tile([1, FD], f32, tag=f"rrow{ic}", name=f"rrow{ic}")
            nc.vector.reciprocal(rrow, d_ps)  # 1/D, queries on free dim
            for t in range(FD // P):
                nc.tensor.matmul(
                    dc_ps[:, t:t + 1],
                    lhsT=rrow[:, t * P:(t + 1) * P],
                    rhs=ones_1, start=True, stop=True,
                )
            rcol = smallp.tile([P, FD // P], f32, tag="rcol", bufs=2)
            nc.vector.tensor_copy(rcol, dc_ps)
            return rcol

        # prelude — only what the first attention iterations need: the
        # ic=0 queries and the first few vt' tiles. The rest is deferred
        # into the attention window. Copies ride the idle ScalarE (qm) and
        # VectorE (vt) so neither queue convoys behind the other.
        for co in range(CI):
            emit_qm(co, 0, act=True)
        for jt in range(4):
            emit_vt(jt, act=False)
        emit_apply(1, ["vec", "pool", "pool", "vec"])

        # ================= query chunk ic=0 =================
        # PV accumulates it-tiles {0,1} in-loop (outp); {2,3} run as a
        # sweep over the persisted pexp, interleaved into ic=1's u-loop
        # (4 matmuls per step) on the vtp banks once vt' production ends.
        # PE stream is skewed: PV(u-1) is emitted after S(u) so PE never
        # waits on the exp round-trip.
        ot01_0 = [
            outp.tile([P, FD], f32, tag=f"ot{t}", name=f"ot{t}_0")
            for t in range(2)
        ]
        for u in range(JT // 2):
            emit_s(0, u)
            if u > 0:
                for t in range(2):
                    emit_pv(0, u - 1, t, ot01_0[t], start=(u == 1),
                            stop=False)
            # software-pipelined production for upcoming iterations
            for jtn in (2 * u + 4, 2 * u + 5):
                if jtn < JT:
                    emit_vt(jtn, act=(jtn % 8 == 7))
            if u == 1:  # second query chunk, needed from ic=1 on
                for co in range(CI):
                    emit_qm(co, 1, act=False)
            if u == 2:
                emit_apply(2, ["pool", "vec", "pool", "vec"])
            if u == 4:
                emit_apply(3, ["pool", "vec", "pool", "vec"])
        gnscope.close()
        for t in range(2):
            emit_pv(0, JT // 2 - 1, t, ot01_0[t], start=False, stop=True)

        # ---- boundary: first ic1 S units keep PE/ScalarE busy while the
        # ic0 denominator chain and epilogue {0,1} run ----
        emit_s(1, 0)
        emit_s(1, 1)
        dps = sxp.tile([P, 2, FD], f32, tag="s", name="d_0")
        dcp = sxp.tile([P, 2, FD], f32, tag="s", name="dc_0")
        rcol0 = emit_dchain(0, dps[0:1, 0, :], dcp[:, 0, 0:FD // P], False)
        for t in range(2):
            emit_epi(0, t, ot01_0[t], rcol0)

        # ================= query chunk ic=1 =================
        ot01_1 = [
            outp.tile([P, FD], f32, tag=f"ot{t}", name=f"ot{t}_1")
            for t in range(2)
        ]
        ot23_0 = [
            vtp.tile([P, FD], f32, tag="vt", name=f"otv{t}_0")
            for t in range(2)
        ]
        ot23_1 = None
        for u in range(2, JT // 2):
            emit_s(1, u)
            if u == 2:
                for t in range(2):
                    emit_pv(1, 0, t, ot01_1[t], start=True, stop=False)
            for t in range(2):
                emit_pv(1, u - 1, t, ot01_1[t], start=False, stop=False)
            if 2 <= u <= 9:
                # ic0 it{2,3} sweep: 4 matmuls per step
                for su in (2 * (u - 2), 2 * (u - 2) + 1):
                    for t in range(2):
                        emit_pv(0, su, 2 + t, ot23_0[t], start=(su == 0),
                                stop=(su == JT // 2 - 1))
            if u == 9:
                for t in range(2):
                    emit_epi(0, 2 + t, ot23_0[t], rcol0)
            if u == 10:
                ot23_1 = [
                    vtp.tile([P, FD], f32, tag="vt", name=f"otv{t}_1")
                    for t in range(2)
                ]
            if u >= 10:
                # ic1 it{2,3} catch-up sweep on the freed vtp banks
                for su in (2 * (u - 10), 2 * (u - 10) + 1):
                    for t in range(2):
                        emit_pv(1, su, 2 + t, ot23_1[t], start=(su == 0),
                                stop=False)
        for t in range(2):
            emit_pv(1, JT // 2 - 1, t, ot01_1[t], start=False, stop=True)
        for su in range(12, JT // 2):
            for t in range(2):
                emit_pv(1, su, 2 + t, ot23_1[t], start=False,
                        stop=(su == JT // 2 - 1))
        dps1 = sxp.tile([P, 2, FD], f32, tag="s", name="d_1")
        dcp1 = sxp.tile([P, 2, FD], f32, tag="s", name="dc_1")
        rcol1 = emit_dchain(1, dps1[0:1, 0, :], dcp1[:, 0, 0:FD // P], False)
        # final epilogues are the kernel tail: fan the scale/add chains
        # across engines (DVE/Act scales; Pool/DVE adds) and batch the out
        # DMAs in it-pairs so the tail drains in ~2 chains instead of 4
        srcs = [ot01_1[0], ot01_1[1], ot23_1[0], ot23_1[1]]
        for pair in range(2):
            otp = outst.tile([P, 2, C], f32, tag="otp")
            for k in range(2):
                t = 2 * pair + k
                it = (FD // P) + t  # ic=1 query tiles
                dst = otp[:, k, :]
                if k == 0:
                    nc.vector.tensor_scalar(
                        dst, srcs[t], rcol1[:, t:t + 1], None, OP.mult
                    )
                    nc.gpsimd.tensor_add(dst, dst, xres_s[:, it, :])
                else:
                    nc.scalar.activation(
                        dst, srcs[t], AF.Copy, scale=rcol1[:, t:t + 1]
                    )
                    nc.vector.tensor_add(dst, dst, xres_s[:, it, :])
            it0 = (FD // P) + 2 * pair
            nc.sync.dma_start(out_t[:, it0:it0 + 2, :], otp)
        ph.close()

    nc.compile()  # bacc passes: wait legalization, event sems, nop fusion
    return nc


_NC = None


def _get_nc():
    global _NC
    if _NC is None:
        _NC = build_bass()
    return _NC


def _prep_core_inputs(x, gn_scale, gn_bias, qw, qb, kw, kb, vw, vb, pw, pb):
    """Build the 8 per-core input maps (host-side sharding / layout prep)."""
    f32 = np.float32
    f64 = np.float64

    def chunkP(a2d):  # [C, M] -> [128, C//128, M]
        Cdim, M = a2d.shape
        return np.ascontiguousarray(
            a2d.reshape(CI, P, M).transpose(1, 0, 2)
        )

    def colsP(v):  # [C] -> [128, CI]
        return np.ascontiguousarray(np.asarray(v, f32).reshape(CI, P).T)

    # host-side weight folds (f64 for exactness)
    qw64, kw64 = np.asarray(qw, f64), np.asarray(kw, f64)
    vw64, pw64 = np.asarray(vw, f64), np.asarray(pw, f64)
    M = qw64.T @ kw64                      # [c, d]: S = xn^T M xn
    Wpv = pw64 @ vw64                      # [o, c]: out = Wpv (xn P)
    u0 = kw64.T @ np.asarray(qb, f64)      # [d]: key-side bias term
    kc0 = pw64 @ np.asarray(vb, f64) + np.asarray(pb, f64)  # [o]

    g_red = np.zeros((P, 8), f32)
    for p in range(P):
        g_red[p, p // 16] = 1.0 / 16.0
    g_bc = np.zeros((8, P), f32)
    for p in range(P):
        g_bc[p // 16, p] = 1.0

    shared = {
        "mT": (chunkP(M.astype(f32)) * WS).astype(F8),
        "wpvT": (chunkP(Wpv.T.astype(f32)) * WS).astype(F8),
        "u0c": colsP(u0.astype(f32)),
        "kc0_bc": np.ascontiguousarray(
            np.broadcast_to(kc0.astype(f32), (P, C))
        ),
        "gns_t": colsP(gn_scale),
        "gnb_t": colsP(gn_bias),
        "g_red": g_red,
        "g_bc": g_bc,
    }

    xf = np.asarray(x, f32).reshape(B, C, N)
    in_maps = []
    for core in range(8):
        b, q = core // 4, core % 4
        # Roll pixels so this core's query quarter starts at pixel 0.
        # GN stats and attention-over-keys are permutation invariant, so
        # keys over rolled pixel order give identical results.
        xi = chunkP(np.roll(xf[b], -q * NQ, axis=1))  # [128, CI, N]
        xq = xf[b][:, q * NQ:(q + 1) * NQ]  # [C, NQ]
        xrT = np.ascontiguousarray(
            xq.T.reshape(IT, P, C).transpose(1, 0, 2)
        )  # [128, IT, C]
        in_maps.append({"x_img": xi, "x_resT": xrT, **shared})
    return in_maps


def _assemble(results):
    """results: list of 8 dicts with out_t [128, IT, C] -> [B, C, H, W]."""
    out = np.empty((B, C, N), np.float32)
    for core in range(8):
        b, q = core // 4, core % 4
        ot = np.asarray(results[core]["out_t"])  # [P, IT, C]
        # i_local = it*P + p ; out[b, :, q*NQ + i_local] = ot[p, it, :]
        blk = ot.transpose(1, 0, 2).reshape(NQ, C)  # [i_local, c]
        out[b, :, q * NQ:(q + 1) * NQ] = blk.T
    return out.reshape(B, C, H, W)


def kernel(**inputs):
    from concourse.bass_utils import run_bass_kernel_spmd

    nc = _get_nc()
    in_maps = _prep_core_inputs(**inputs)
    res = run_bass_kernel_spmd(nc, in_maps, core_ids=list(range(8)))
    return _assemble(res.results)


if __name__ == "__main__":
    nc = build_bass()
    print("built OK")


# revision 17
# speedup vs baseline: 1.3013x; 1.0143x over previous
"""AttnBlock (GroupNorm -> QKV 1x1 -> HxW self-attention -> proj -> residual)
as a Bass/Tile kernel on 8 TRN2 NeuronCores.

Sharding: data-parallel over batch B=2 and sequence-parallel over HW
quarters (4 cores per image, 1024 queries each). Each core redundantly
computes GroupNorm + full-image keys/values so there is no cross-core
communication. The host rolls the pixel axis per core so each core's
query quarter starts at pixel 0 (GN stats and attention-over-keys are
permutation invariant), letting all cores run one SPMD program.

Algebraic folds (host side) eliminate whole on-device phases:
  S   = q.k = (Wq xn + qb).(Wk xn + kb)
      = xn^T (Wq^T Wk) xn + (Wk^T qb).xn_j + [terms const over keys j,
        which softmax cancels]
    so with M = Wq^T Wk and u0 = Wk^T qb folded on host:
        qm = M^T xn_q + u0   (replaces the Q AND K projections)
        S  = qm^T xn         (keys are raw xn -- no K tensor at all)
  out = Wp (V P) + pb, V = Wv xn + vb, P = softmax rows
      = (Wp Wv) (xn P) + (pw vb + pb)
    so with Wpv = Wp Wv and kc0 = pw vb + pb folded on host:
        vt' = xn^T Wpv^T     (one value pass instead of V-proj + out-proj)
        out^T[i,:] = sum_j P[j,i] vt'[j,:]  -- the PV matmul emits the
        FINAL projected output directly in [query, channel] layout, so
        the epilogue is just a per-partition 1/D scale + residual add.
This removes ~1.3G MACs/core of matmul and ~100 PSUM->SBUF copy ops.

Precision: matmul operands in fp8e4 (E4M3) with DoubleRow perf mode;
fp32 PSUM accumulation. Folded weights are pre-scaled x256 on the host
(fp8 normal range); the 1/256 folds into existing psum->SBUF copies.
Softmax skips max-subtraction (logits in ~[-1.5, 1.5] by construction);
the softmax division is a per-partition scale in the epilogue.

Engine balance: exp runs in batched [128, 2x512] ops on ScalarE; the
GN-apply cast and the PSUM->SBUF copies are split across Vector/Scalar/
GpSimd; residual adds ride the otherwise idle GpSimd engine.
"""

import sys

sys.path.insert(0, "/opt/trn_rl_repo")

import numpy as np
import ml_dtypes

B, C, H, W = 2, 512, 64, 64
N = H * W            # 4096 pixels per image
NQ = N // 4          # 1024 queries per core
CI = C // 128        # 4 channel chunks of 128
NUM_GROUPS = 32
EPS = 1e-6
P = 128
FD = 512             # matmul moving free dim
JT = N // P          # 32 key tiles
IC = NQ // FD        # 2 query chunks of 512
IT = NQ // P         # 8 query tiles of 128
SCALE = float(C) ** -0.5
WS = 256.0           # host-side weight pre-scale (keeps fp8e4 in normal range)

F8 = ml_dtypes.float8_e4m3


def build_bass():
    import concourse.bass as bass
    import concourse.tile as tile
    import concourse.mybir as mybir
    from concourse import bacc
    from contextlib import ExitStack

    f32 = mybir.dt.float32
    f8 = mybir.dt.float8e4
    AF = mybir.ActivationFunctionType
    OP = mybir.AluOpType
    DR = mybir.MatmulPerfMode.DoubleRow

    nc = bacc.Bacc("TRN2")

    # ---------------- DRAM I/O ----------------
    x_img = nc.dram_tensor("x_img", [P, CI, N], f32, kind="ExternalInput")
    x_resT = nc.dram_tensor("x_resT", [P, IT, C], f32, kind="ExternalInput")
    mT = nc.dram_tensor("mT", [P, CI, C], f8, kind="ExternalInput")
    wpvT = nc.dram_tensor("wpvT", [P, CI, C], f8, kind="ExternalInput")
    u0c = nc.dram_tensor("u0c", [P, CI], f32, kind="ExternalInput")
    kc0_bc = nc.dram_tensor("kc0_bc", [P, C], f32, kind="ExternalInput")
    gns_t = nc.dram_tensor("gns_t", [P, CI], f32, kind="ExternalInput")
    gnb_t = nc.dram_tensor("gnb_t", [P, CI], f32, kind="ExternalInput")
    g_red = nc.dram_tensor("g_red", [P, 8], f32, kind="ExternalInput")
    g_bc = nc.dram_tensor("g_bc", [8, P], f32, kind="ExternalInput")
    out_t = nc.dram_tensor("out_t", [P, IT, C], f32, kind="ExternalOutput")

    with tile.TileContext(nc) as tc, ExitStack() as top:
        consts = top.enter_context(tc.tile_pool(name="consts", bufs=1))
        big = top.enter_context(tc.tile_pool(name="big", bufs=1))
        smallp = top.enter_context(tc.tile_pool(name="smallp", bufs=1))
        outst = top.enter_context(tc.tile_pool(name="outst", bufs=4))

        # tiny GroupNorm constants FIRST — the per-chunk reduce chain needs
        # them early and they'd otherwise queue behind 8MB of x (their cost
        # to x's arrival is negligible)
        gns_s = consts.tile([P, CI], f32)
        gnb_s = consts.tile([P, CI], f32)
        nc.sync.dma_start(gns_s, gns_t[:])
        nc.sync.dma_start(gnb_s, gnb_t[:])
        gr_s = consts.tile([P, 8], f32)
        gb_s = consts.tile([8, P], f32)
        nc.sync.dma_start(gr_s, g_red[:])
        nc.sync.dma_start(gb_s, g_bc[:])

        # x next — GroupNorm stats are the critical-path head
        gnscope = ExitStack()
        xresid = gnscope.enter_context(tc.tile_pool(name="xresid", bufs=1))
        xr = [xresid.tile([P, N], f32, name=f"xr{ci}") for ci in range(CI)]
        for ci in range(CI):
            for h in range(4):
                nc.sync.dma_start(
                    xr[ci][:, h * 1024:(h + 1) * 1024],
                    x_img[:, ci, h * 1024:(h + 1) * 1024],
                )

        # ---- remaining constants (first used ~25us in) ----
        m_s = consts.tile([P, CI, C], f8)
        wpv_s = consts.tile([P, CI, C], f8)
        nc.sync.dma_start(m_s, mT[:])
        nc.sync.dma_start(wpv_s, wpvT[:])
        u0_s = consts.tile([P, CI], f32)
        nc.sync.dma_start(u0_s, u0c[:])
        kc0_s = consts.tile([P, C], f32)
        nc.sync.dma_start(kc0_s, kc0_bc[:])
        # padded to 16 so the DoubleRow pair-plane stride is 16B (%16 rule)
        # (memsets on GpSimd: VectorE's in-order queue must stay clear for
        # the GroupNorm stats stream)
        ones2 = consts.tile([P, 2, 16], f8)
        nc.gpsimd.memset(ones2, 1.0)
        ones_1 = consts.tile([1, 1], f32)
        nc.gpsimd.memset(ones_1, 1.0)
        eps8 = consts.tile([8, 1], f32)
        nc.gpsimd.memset(eps8, EPS)

        xres_s = big.tile([P, IT, C], f32)
        nc.sync.dma_start(xres_s, x_resT[:])

        # residual pre-adds (x + kc0 const row) on the otherwise-idle
        # GpSimd engine (SBUF-only); runs in the DMA-bound prelude
        for it in range(IT):
            nc.gpsimd.tensor_add(xres_s[:, it, :], xres_s[:, it, :], kc0_s)

        # big persistent tensors (fp8)
        xn = big.tile([P, CI, N], f8)            # normalized x (keys AND gn)
        vt_sb = big.tile([P, JT, C], f8)         # vt' = xn^T Wpv^T, [j, o]
        qm_sb = big.tile([P, CI, NQ], f8)        # qm = M^T xn_q + u0, [d, i]
        # exp(S), double-buffered per query-half, persisted so the
        # softmax-denominator reduction and the PV sweeps can run off the
        # hot loop without write-after-read stalls between halves
        pexpall = big.tile([P, IC, JT // 2, 2, FD], f8)

        # =============== Phase 1: GroupNorm ===============
        gnw = gnscope.enter_context(tc.tile_pool(name="gnw", bufs=1))
        gnps_scope = ExitStack()
        gnps = gnps_scope.enter_context(
            tc.tile_pool(name="gnps", bufs=1, space="PSUM")
        )

        mv2 = gnw.tile([P, 2 * CI], f32)  # per-channel (mean, m2) per chunk
        gps = gnps.tile([8, 2 * CI], f32, tag="g")
        gst = gnw.tile([8, 2 * CI], f32)
        bcps = gnps.tile([P, 2 * CI], f32, tag="bc")
        a_all = gnw.tile([P, CI], f32)
        b_all = gnw.tile([P, CI], f32)
        # Pass 1 — the DVE queue carries ONLY stats (bn_stats/bn_aggr), so
        # it tracks the x DMA stream with no convoying; the ScalarE chain
        # up to sqrt(var) runs per-chunk in parallel. The reciprocals (DVE)
        # are deferred to pass 2 so they never block later chunks' stats.
        for ci in range(CI):
            bnst = gnw.tile([P, 8, 6], f32, tag="bnst", bufs=2)
            for s in range(8):
                nc.vector.bn_stats(
                    bnst[:, s, :], xr[ci][:, s * 512:(s + 1) * 512]
                )
            nc.vector.bn_aggr(mv2[:, 2 * ci:2 * ci + 2], bnst)
            mu = mv2[:, 2 * ci:2 * ci + 1]
            m2 = mv2[:, 2 * ci + 1:2 * ci + 2]
            sq = gnw.tile([P, 1], f32, tag="sq", bufs=2)
            nc.scalar.activation(sq, mu, AF.Copy, scale=mu)      # mean^2
            nc.scalar.activation(m2, sq, AF.Identity, bias=m2)   # 2nd moment
            nc.tensor.matmul(
                gps[:, 2 * ci:2 * ci + 2], lhsT=gr_s,
                rhs=mv2[:, 2 * ci:2 * ci + 2], start=True, stop=True,
            )
            gmu = gst[:, 2 * ci:2 * ci + 1]
            gm2 = gst[:, 2 * ci + 1:2 * ci + 2]
            nc.scalar.copy(gst[:, 2 * ci:2 * ci + 2], gps[:, 2 * ci:2 * ci + 2])
            gsq = gnw.tile([8, 1], f32, tag="gsq", bufs=2)
            nc.scalar.activation(gsq, gmu, AF.Copy, scale=gmu)   # gmean^2
            nc.scalar.activation(gm2, gsq, AF.Identity, scale=-1.0, bias=gm2)
            nc.scalar.activation(gm2, gm2, AF.Sqrt, bias=eps8)   # std
        # Pass 2 — reciprocals and the (a, b) affine coefficients
        for ci in range(CI):
            gm2 = gst[:, 2 * ci + 1:2 * ci + 2]
            nc.vector.reciprocal(gm2, gm2)                       # rstd
            nc.tensor.matmul(
                bcps[:, 2 * ci:2 * ci + 2], lhsT=gb_s,
                rhs=gst[:, 2 * ci:2 * ci + 2], start=True, stop=True,
            )
            a = a_all[:, ci:ci + 1]
            b = b_all[:, ci:ci + 1]
            chp = gnw.tile([P, 2], f32, tag="chp", bufs=2)
            nc.scalar.copy(chp, bcps[:, 2 * ci:2 * ci + 2])
            nc.scalar.activation(a, gns_s[:, ci:ci + 1], AF.Copy,
                                 scale=chp[:, 1:2])              # rstd*gns
            tmpc = gnw.tile([P, 1], f32, tag="tmpc", bufs=2)
            nc.scalar.activation(tmpc, chp[:, 0:1], AF.Copy, scale=a)
            nc.scalar.activation(b, tmpc, AF.Identity, scale=-1.0,
                                 bias=gnb_s[:, ci:ci + 1])

        def emit_apply(h, engines):
            # xn[:, ci, h-quarter] = a*x + b, split across engines
            for ci in range(CI):
                dst = xn[:, ci, h * 1024:(h + 1) * 1024]
                src = xr[ci][:, h * 1024:(h + 1) * 1024]
                eng = engines[ci % len(engines)]
                if eng == "act":
                    nc.scalar.activation(
                        dst, src, AF.Identity,
                        bias=b_all[:, ci:ci + 1], scale=a_all[:, ci:ci + 1],
                    )
                elif eng == "pool":
                    nc.gpsimd.tensor_scalar(
                        dst, src,
                        a_all[:, ci:ci + 1], b_all[:, ci:ci + 1],
                        OP.mult, OP.add,
                    )
                else:
                    nc.vector.tensor_scalar(
                        dst, src,
                        a_all[:, ci:ci + 1], b_all[:, ci:ci + 1],
                        OP.mult, OP.add,
                    )

        # quarter 0 first (it holds the queries + first key tiles); fan the
        # chunk casts across all three elementwise engines. ci3's (a,b)
        # lands last, so it gets the otherwise-idle ScalarE.
        emit_apply(0, ["vec", "pool", "vec", "act"])

        # psum pools for attention (8 banks): s-tiles 2x2, vt'/qm 2x1,
        # out^T accumulators 2x1; the GN psum banks must be gone first
        gnps_scope.close()
        ph = ExitStack()
        sxp = ph.enter_context(tc.tile_pool(name="sxp", bufs=2, space="PSUM"))
        vtp = ph.enter_context(tc.tile_pool(name="vtp", bufs=2, space="PSUM"))
        outp = ph.enter_context(tc.tile_pool(name="outp", bufs=1, space="PSUM"))

        def emit_qm(co, icq, act=True):
            # qm[d-block co, i-chunk icq] = M^T xn_q + u0
            ps = vtp.tile([P, FD], f32, tag="vt", name=f"qm{co}_{icq}")
            for ep in range(CI // 2):
                nc.tensor.matmul(
                    ps,
                    lhsT=m_s[:, 2 * ep:2 * ep + 2, co * P:(co + 1) * P],
                    rhs=xn[:, 2 * ep:2 * ep + 2, icq * FD:(icq + 1) * FD],
                    start=(ep == 0),
                    stop=(ep == CI // 2 - 1),
                    perf_mode=DR,
                )
            if act:
                nc.scalar.activation(
                    qm_sb[:, co, icq * FD:(icq + 1) * FD], ps, AF.Identity,
                    bias=u0_s[:, co:co + 1], scale=1.0 / WS,
                )
            else:
                nc.vector.tensor_scalar(
                    qm_sb[:, co, icq * FD:(icq + 1) * FD], ps,
                    1.0 / WS, u0_s[:, co:co + 1], OP.mult, OP.add,
                )

        def emit_vt(jt, act=False):
            # vt'[j-tile jt, :] = xn^T Wpv^T
            ps = vtp.tile([P, FD], f32, tag="vt", name=f"vt{jt}")
            for ep in range(CI // 2):
                nc.tensor.matmul(
                    ps,
                    lhsT=xn[:, 2 * ep:2 * ep + 2, jt * P:(jt + 1) * P],
                    rhs=wpv_s[:, 2 * ep:2 * ep + 2, :],
                    start=(ep == 0),
                    stop=(ep == CI // 2 - 1),
                    perf_mode=DR,
                )
            if act:
                nc.scalar.activation(
                    vt_sb[:, jt, :], ps, AF.Copy, scale=1.0 / WS
                )
            else:
                nc.vector.tensor_scalar(
                    vt_sb[:, jt, :], ps, 1.0 / WS, None, OP.mult
                )

        def emit_s(ic, u):
            # S^T for key tiles (2u, 2u+1) x query chunk ic, then one
            # batched exp into the persisted pexp buffer
            s2 = sxp.tile([P, 2, FD], f32, tag="s", name=f"s{u}_{ic}")
            for t in range(2):
                jt = 2 * u + t
                for ep in range(CI // 2):
                    nc.tensor.matmul(
                        s2[:, t, :],
                        lhsT=xn[:, 2 * ep:2 * ep + 2, jt * P:(jt + 1) * P],
                        rhs=qm_sb[:, 2 * ep:2 * ep + 2, ic * FD:(ic + 1) * FD],
                        start=(ep == 0),
                        stop=(ep == CI // 2 - 1),
                        perf_mode=DR,
                    )
            nc.scalar.activation(pexpall[:, ic, u, :, :], s2, AF.Exp,
                                 scale=SCALE)

        def emit_pv(ic, u, t, dst, start, stop):
            nc.tensor.matmul(
                dst,
                lhsT=pexpall[:, ic, u, :, t * P:(t + 1) * P],
                rhs=vt_sb[:, 2 * u:2 * u + 2, :],
                start=start, stop=stop, perf_mode=DR,
            )

        def emit_epi(ic, t, src, rcol):
            it = ic * (FD // P) + t
            ot = outst.tile([P, C], f32, tag="ot")
            nc.vector.tensor_scalar(ot, src, rcol[:, t:t + 1], None, OP.mult)
            nc.gpsimd.tensor_add(ot, ot, xres_s[:, it, :])
            nc.sync.dma_start(out_t[:, it, :], ot)

        def emit_dchain(ic, d_ps, dc_ps, in_loop):
            if not in_loop:
                for u in range(JT // 2):
                    nc.tensor.matmul(
                        d_ps, lhsT=ones2[:, :, 0:1],
                        rhs=pexpall[:, ic, u, :, :],
                        start=(u == 0), stop=(u == JT // 2 - 1), perf_mode=DR,
                    )
            rrow = smallp.tile([1, FD], f32, tag=f"rrow{ic}", name=f"rrow{ic}")
            nc.vector.reciprocal(rrow, d_ps)  # 1/D, queries on free dim
            for t in range(FD // P):
                nc.tensor.matmul(
                    dc_ps[:, t:t + 1],
                    lhsT=rrow[:, t * P:(t + 1) * P],
                    rhs=ones_1, start=True, stop=True,
                )
            rcol = smallp.tile([P, FD // P], f32, tag="rcol", bufs=2)
            nc.vector.tensor_copy(rcol, dc_ps)
            return rcol

        # prelude — only what the first attention iterations need: the
        # ic=0 queries and the first few vt' tiles. The rest is deferred
        # into the attention window. Copies ride the idle ScalarE (qm) and
        # VectorE (vt) so neither queue convoys behind the other.
        for co in range(CI):
            emit_qm(co, 0, act=True)
        for jt in range(4):
            emit_vt(jt, act=False)
        emit_apply(1, ["vec", "pool", "pool", "vec"])

        # ================= query chunk ic=0 =================
        # PV accumulates it-tiles {0,1} in-loop (outp); {2,3} run as a
        # sweep over the persisted pexp, interleaved into ic=1's u-loop
        # (4 matmuls per step) on the vtp banks once vt' production ends.
        # PE stream is skewed: PV(u-1) is emitted after S(u) so PE never
        # waits on the exp round-trip.
        ot01_0 = [
            outp.tile([P, FD], f32, tag=f"ot{t}", name=f"ot{t}_0")
            for t in range(2)
        ]
        for u in range(JT // 2):
            emit_s(0, u)
            if u > 0:
                for t in range(2):
                    emit_pv(0, u - 1, t, ot01_0[t], start=(u == 1),
                            stop=False)
            # software-pipelined production for upcoming iterations
            for jtn in (2 * u + 4, 2 * u + 5):
                if jtn < JT:
                    emit_vt(jtn, act=(jtn % 8 == 7))
            if u == 1:  # second query chunk, needed from ic=1 on
                for co in range(CI):
                    emit_qm(co, 1, act=False)
            if u == 2:
                emit_apply(2, ["pool", "vec", "pool", "vec"])
            if u == 4:
                emit_apply(3, ["pool", "vec", "pool", "vec"])
        gnscope.close()
        for t in range(2):
            emit_pv(0, JT // 2 - 1, t, ot01_0[t], start=False, stop=True)

        # ---- boundary: four ic1 S units keep ScalarE fed while the ic0
        # denominator chain and epilogue {0,1} run on PE/DVE ----
        for u in range(4):
            emit_s(1, u)
        dps = sxp.tile([P, 2, FD], f32, tag="s", name="d_0")
        dcp = sxp.tile([P, 2, FD], f32, tag="s", name="dc_0")
        rcol0 = emit_dchain(0, dps[0:1, 0, :], dcp[:, 0, 0:FD // P], False)
        for t in range(2):
            emit_epi(0, t, ot01_0[t], rcol0)

        # ================= query chunk ic=1 =================
        # (S runs 4 ahead of PV01 here, covering the boundary burst)
        ot01_1 = [
            outp.tile([P, FD], f32, tag=f"ot{t}", name=f"ot{t}_1")
            for t in range(2)
        ]
        ot23_0 = [
            vtp.tile([P, FD], f32, tag="vt", name=f"otv{t}_0")
            for t in range(2)
        ]
        ot23_1 = None
        dps1 = None
        for u in range(4, JT // 2):
            emit_s(1, u)
            for t in range(2):
                emit_pv(1, u - 4, t, ot01_1[t], start=(u == 4), stop=False)
            if u <= 11:
                # ic0 it{2,3} sweep: 4 matmuls per step
                for su in (2 * (u - 4), 2 * (u - 4) + 1):
                    for t in range(2):
                        emit_pv(0, su, 2 + t, ot23_0[t], start=(su == 0),
                                stop=(su == JT // 2 - 1))
            if u == 11:
                for t in range(2):
                    emit_epi(0, 2 + t, ot23_0[t], rcol0)
            if u == 12:
                ot23_1 = [
                    vtp.tile([P, FD], f32, tag="vt", name=f"otv{t}_1")
                    for t in range(2)
                ]
            if u >= 12:
                # ic1 it{2,3} catch-up sweep on the freed vtp banks
                for su in (2 * (u - 12), 2 * (u - 12) + 1):
                    for t in range(2):
                        emit_pv(1, su, 2 + t, ot23_1[t], start=(su == 0),
                                stop=False)
            if u == JT // 2 - 1:
                # denominator head (keys 0..13) overlaps the last exp ops
                dps1 = sxp.tile([P, 2, FD], f32, tag="s", name="d_1")
                for su in range(JT // 2 - 2):
                    nc.tensor.matmul(
                        dps1[0:1, 0, :], lhsT=ones2[:, :, 0:1],
                        rhs=pexpall[:, 1, su, :, :],
                        start=(su == 0), stop=False, perf_mode=DR,
                    )
        # ---- ic1 tail: denominator finish -> rcol asap, epilogues fanned
        # across engines, PV post-sweeps between ----
        for su in range(JT // 2 - 2, JT // 2):
            nc.tensor.matmul(
                dps1[0:1, 0, :], lhsT=ones2[:, :, 0:1],
                rhs=pexpall[:, 1, su, :, :],
                start=False, stop=(su == JT // 2 - 1), perf_mode=DR,
            )
        rrow1 = smallp.tile([1, FD], f32, tag="rrow1", name="rrow1")
        nc.vector.reciprocal(rrow1, dps1[0:1, 0, :])
        for u in range(JT // 2 - 4, JT // 2):
            for t in range(2):
                emit_pv(1, u, t, ot01_1[t], start=False,
                        stop=(u == JT // 2 - 1))
        dcp1 = sxp.tile([P, 2, FD], f32, tag="s", name="dc_1")
        dc1 = dcp1[:, 0, 0:FD // P]
        for t in range(FD // P):
            nc.tensor.matmul(
                dc1[:, t:t + 1], lhsT=rrow1[:, t * P:(t + 1) * P],
                rhs=ones_1, start=True, stop=True,
            )
        rcol1 = smallp.tile([P, FD // P], f32, tag="rcol", bufs=2)
        nc.vector.tensor_copy(rcol1, dc1)

        # final epilogues: scale/add chains across engines (DVE/Act scales;
        # Pool/DVE adds), out DMAs batched in it-pairs
        def emit_epi_pair(pair, srcs, rcol):
            otp = outst.tile([P, 2, C], f32, tag="otp")
            for k in range(2):
                t = 2 * pair + k
                it = (FD // P) + t  # ic=1 query tiles
                dst = otp[:, k, :]
                if k == 0:
                    nc.vector.tensor_scalar(
                        dst, srcs[t], rcol[:, t:t + 1], None, OP.mult
                    )
                    nc.gpsimd.tensor_add(dst, dst, xres_s[:, it, :])
                else:
                    nc.scalar.activation(
                        dst, srcs[t], AF.Copy, scale=rcol[:, t:t + 1]
                    )
                    nc.vector.tensor_add(dst, dst, xres_s[:, it, :])
            it0 = (FD // P) + 2 * pair
            nc.sync.dma_start(out_t[:, it0:it0 + 2, :], otp)

        srcs = [ot01_1[0], ot01_1[1], None, None]
        emit_epi_pair(0, srcs, rcol1)
        for su in range(8, JT // 2):
            for t in range(2):
                emit_pv(1, su, 2 + t, ot23_1[t], start=False,
                        stop=(su == JT // 2 - 1))
        srcs = [None, None, ot23_1[0], ot23_1[1]]
        emit_epi_pair(1, srcs, rcol1)
        ph.close()

    nc.compile()  # bacc passes: wait legalization, event sems, nop fusion
    return nc


_NC = None


def _get_nc():
    global _NC
    if _NC is None:
        _NC = build_bass()
    return _NC


def _prep_core_inputs(x, gn_scale, gn_bias, qw, qb, kw, kb, vw, vb, pw, pb):
    """Build the 8 per-core input maps (host-side sharding / layout prep)."""
    f32 = np.float32
    f64 = np.float64

    def chunkP(a2d):  # [C, M] -> [128, C//128, M]
        Cdim, M = a2d.shape
        return np.ascontiguousarray(
            a2d.reshape(CI, P, M).transpose(1, 0, 2)
        )

    def colsP(v):  # [C] -> [128, CI]
        return np.ascontiguousarray(np.asarray(v, f32).reshape(CI, P).T)

    # host-side weight folds (f64 for exactness)
    qw64, kw64 = np.asarray(qw, f64), np.asarray(kw, f64)
    vw64, pw64 = np.asarray(vw, f64), np.asarray(pw, f64)
    M = qw64.T @ kw64                      # [c, d]: S = xn^T M xn
    Wpv = pw64 @ vw64                      # [o, c]: out = Wpv (xn P)
    u0 = kw64.T @ np.asarray(qb, f64)      # [d]: key-side bias term
    kc0 = pw64 @ np.asarray(vb, f64) + np.asarray(pb, f64)  # [o]

    g_red = np.zeros((P, 8), f32)
    for p in range(P):
        g_red[p, p // 16] = 1.0 / 16.0
    g_bc = np.zeros((8, P), f32)
    for p in range(P):
        g_bc[p // 16, p] = 1.0

    shared = {
        "mT": (chunkP(M.astype(f32)) * WS).astype(F8),
        "wpvT": (chunkP(Wpv.T.astype(f32)) * WS).astype(F8),
        "u0c": colsP(u0.astype(f32)),
        "kc0_bc": np.ascontiguousarray(
            np.broadcast_to(kc0.astype(f32), (P, C))
        ),
        "gns_t": colsP(gn_scale),
        "gnb_t": colsP(gn_bias),
        "g_red": g_red,
        "g_bc": g_bc,
    }

    xf = np.asarray(x, f32).reshape(B, C, N)
    in_maps = []
    for core in range(8):
        b, q = core // 4, core % 4
        # Roll pixels so this core's query quarter starts at pixel 0.
        # GN stats and attention-over-keys are permutation invariant, so
        # keys over rolled pixel order give identical results.
        xi = chunkP(np.roll(xf[b], -q * NQ, axis=1))  # [128, CI, N]
        xq = xf[b][:, q * NQ:(q + 1) * NQ]  # [C, NQ]
        xrT = np.ascontiguousarray(
            xq.T.reshape(IT, P, C).transpose(1, 0, 2)
        )  # [128, IT, C]
        in_maps.append({"x_img": xi, "x_resT": xrT, **shared})
    return in_maps


def _assemble(results):
    """results: list of 8 dicts with out_t [128, IT, C] -> [B, C, H, W]."""
    out = np.empty((B, C, N), np.float32)
    for core in range(8):
        b, q = core // 4, core % 4
        ot = np.asarray(results[core]["out_t"])  # [P, IT, C]
        # i_local = it*P + p ; out[b, :, q*NQ + i_local] = ot[p, it, :]
        blk = ot.transpose(1, 0, 2).reshape(NQ, C)  # [i_local, c]
        out[b, :, q * NQ:(q + 1) * NQ] = blk.T
    return out.reshape(B, C, H, W)


def kernel(**inputs):
    from concourse.bass_utils import run_bass_kernel_spmd

    nc = _get_nc()
    in_maps = _prep_core_inputs(**inputs)
    res = run_bass_kernel_spmd(nc, in_maps, core_ids=list(range(8)))
    return _assemble(res.results)


if __name__ == "__main__":
    nc = build_bass()
    print("built OK")
